# revision 36
# baseline (speedup 1.0000x reference)
"""Trainium2 Bass kernel for a local-attention transformer block.

Computes, per batch element (one NeuronCore each, 8 cores):
  ss = silu(t_emb) @ time_w + time_b ;  scale, shift = split(ss)
  y  = LN(x) * g1*(1+scale) + (b1*(1+scale)+shift)   (folded host-side into qkv W/b)
  q,k,v = y @ qkv_w + qkv_b  (heads=8, d=64)
  attn: each 128-token window attends to [prev|cur|next] windows
  x1 = x + attn @ proj_w + proj_b
  out = x1 + gelu(LN2(x1) @ w1 + b1') @ w2 + b2       (ln2 g/b host-folded into w1/b1)

Key layout/engine choices (v2):
  - 3-phase software pipeline per 512-token group g: A(g)=LN1+QKV,
    B(g-1)=attention, C(g-2)=proj+MLP; keeps the PE continuously fed.
  - Transposes (y, h2 -> feature-major) via PE matmul against an fp16
    identity (4 per PSUM bank, single evac copy), not DMA xbar.
  - LN rstd = exp(-0.5*ln(var+eps)) so ACT alternates between only the
    natural_log_exp and gelu table sets (2 loads/step instead of ~7).
  - Softmax denominators via ones-column folded into v (row 64 of the AV
    PSUM); normalization broadcast across partitions via a K=2 mask
    matmul instead of DRAM round trips.
  - All matmuls fp16 inputs / fp32 PSUM accumulation.
"""

import numpy as np
from contextlib import ExitStack

import concourse.bass as bass
import concourse.tile as tile
from concourse import bacc, mybir, masks
from concourse import bass_utils

F32 = mybir.dt.float32
F16 = mybir.dt.float16
AF = mybir.ActivationFunctionType
AL = mybir.AluOpType

DIM = 512
HEADS = 8
HD = 64
FF = 2048
WIN = 128
B = 8
NTOK = 8192
EPS = 1e-5
GRP = 512  # tokens per group (4 windows)


def _bcast_row(dram_ap, offset, n):
    """AP reading dram vector [n] broadcast across 128 partitions."""
    return bass.AP(tensor=dram_ap.tensor, offset=offset, ap=[[0, 128], [1, n]])


def _col_view(dram_ap, offset, ncol):
    """AP reading dram vector [128*ncol] as [128, ncol] feature-major columns."""
    return bass.AP(tensor=dram_ap.tensor, offset=offset, ap=[[1, 128], [128, ncol]])


def build(n_tok=NTOK):
    n_groups = n_tok // GRP
    n_blocks = n_tok // WIN
    nc = bacc.Bacc("TRN2", target_bir_lowering=False, debug=False)

    x_d = nc.dram_tensor("x", [n_tok, DIM], F32, kind="ExternalInput")
    qkvw_d = nc.dram_tensor("qkvw", [DIM, 3 * DIM], F16, kind="ExternalInput")
    qkvb_d = nc.dram_tensor("qkvb", [3 * DIM], F32, kind="ExternalInput")
    projw_d = nc.dram_tensor("projw", [DIM, DIM], F16, kind="ExternalInput")
    projb_d = nc.dram_tensor("projb", [DIM], F32, kind="ExternalInput")
    w1_d = nc.dram_tensor("w1", [DIM, FF], F16, kind="ExternalInput")
    b1_d = nc.dram_tensor("b1", [FF], F32, kind="ExternalInput")
    w2_d = nc.dram_tensor("w2", [FF, DIM], F16, kind="ExternalInput")
    b2_d = nc.dram_tensor("b2", [DIM], F32, kind="ExternalInput")
    out_d = nc.dram_tensor("out", [n_tok, DIM], F32, kind="ExternalOutput")

    with tile.TileContext(nc) as tc:
        with ExitStack() as ctx:
            consts = ctx.enter_context(tc.tile_pool(name="consts", bufs=1))
            xp = ctx.enter_context(tc.tile_pool(name="xp", bufs=3))
            tp = ctx.enter_context(tc.tile_pool(name="tp", bufs=2))
            yp = ctx.enter_context(tc.tile_pool(name="yp", bufs=1))
            ytp = ctx.enter_context(tc.tile_pool(name="ytp", bufs=1))
            qp = ctx.enter_context(tc.tile_pool(name="qp", bufs=2))
            kp = ctx.enter_context(tc.tile_pool(name="kp", bufs=3))
            vp = ctx.enter_context(tc.tile_pool(name="vp", bufs=3))
            ep = ctx.enter_context(tc.tile_pool(name="ep", bufs=3))
            ap_ = ctx.enter_context(tc.tile_pool(name="ap", bufs=2))
            rp = ctx.enter_context(tc.tile_pool(name="rp", bufs=1))
            x1p = ctx.enter_context(tc.tile_pool(name="x1p", bufs=1))
            h2p = ctx.enter_context(tc.tile_pool(name="h2p", bufs=1))
            h2tp = ctx.enter_context(tc.tile_pool(name="h2tp", bufs=1))
            gp = ctx.enter_context(tc.tile_pool(name="gp", bufs=1))
            op = ctx.enter_context(tc.tile_pool(name="op", bufs=1))
            ps_g = ctx.enter_context(tc.tile_pool(name="ps_g", bufs=2, space="PSUM"))
            ps_tp = ctx.enter_context(tc.tile_pool(name="ps_tp", bufs=2, space="PSUM"))
            ps_s = ctx.enter_context(tc.tile_pool(name="ps_s", bufs=2, space="PSUM"))
            ps_a = ctx.enter_context(tc.tile_pool(name="ps_a", bufs=2, space="PSUM"))

            # ---- x prefetch (group 0 first, ahead of the weight DMAs) ----
            xtiles = {}

            def load_x(g):
                ts_ = []
                for t in range(4):
                    xt = xp.tile([128, DIM], F32, name=f"x_{g}_{t}", tag=f"x{t}")
                    nc.sync.dma_start(xt[:],
                                      x_d[(g * 4 + t) * 128:(g * 4 + t + 1) * 128, :])
                    ts_.append(xt)
                xtiles[g] = ts_

            load_x(0)

            # ---- constants ----
            qkvw_sb = []
            for c in range(4):
                t = consts.tile([128, 3 * DIM], F16, name=f"qkvw{c}", tag=f"qkvw{c}")
                nc.sync.dma_start(t[:], qkvw_d[c * 128:(c + 1) * 128, :])
                qkvw_sb.append(t)
            projw_sb = []
            for c in range(4):
                t = consts.tile([128, DIM], F16, name=f"projw{c}", tag=f"projw{c}")
                nc.sync.dma_start(t[:], projw_d[c * 128:(c + 1) * 128, :])
                projw_sb.append(t)
            w1_sb = []
            for c in range(4):
                t = consts.tile([128, FF], F16, name=f"w1_{c}", tag=f"w1_{c}")
                nc.sync.dma_start(t[:], w1_d[c * 128:(c + 1) * 128, :])
                w1_sb.append(t)
            w2_sb = []
            for f in range(16):
                t = consts.tile([128, DIM], F16, name=f"w2_{f}", tag=f"w2_{f}")
                nc.sync.dma_start(t[:], w2_d[f * 128:(f + 1) * 128, :])
                w2_sb.append(t)

            projb_bc = consts.tile([128, DIM], F32, name="projb_bc")
            nc.sync.dma_start(projb_bc[:], _bcast_row(projb_d.ap(), 0, DIM))
            b2_bc = consts.tile([128, DIM], F32, name="b2_bc")
            nc.sync.dma_start(b2_bc[:], _bcast_row(b2_d.ap(), 0, DIM))
            vb_bc = consts.tile([128, DIM], F32, name="vb_bc")
            nc.sync.dma_start(vb_bc[:], _bcast_row(qkvb_d.ap(), 2 * DIM, DIM))
            qkb_sb = consts.tile([128, 8], F32, name="qkb_sb")
            nc.sync.dma_start(qkb_sb[:], _col_view(qkvb_d.ap(), 0, 8))
            b1_sb = consts.tile([128, 16], F32, name="b1_sb")
            nc.sync.dma_start(b1_sb[:], _col_view(b1_d.ap(), 0, 16))
            eps_t = consts.tile([128, 1], F32, name="eps_t")
            nc.vector.memset(eps_t[:], EPS)
            ident = consts.tile([128, 128], F16, name="ident")
            masks.make_identity(nc, ident[:])
            # half-masks: [1,128] fp32 selecting partitions 0-63 / 64-127
            mask_lo = consts.tile([1, 128], F32, name="mask_lo")
            nc.vector.memset(mask_lo[:], 0.0)
            nc.vector.memset(mask_lo[0:1, 0:64], 1.0)
            mask_hi = consts.tile([1, 128], F32, name="mask_hi")
            nc.vector.memset(mask_hi[:], 0.0)
            nc.vector.memset(mask_hi[0:1, 64:128], 1.0)

            stages = {}

            def _rstd4(mvs, tagpfx):
                """rstd[128,4] = 1/sqrt(var+eps) for 4 tiles, DVE-only.

                u = fast_reciprocal(v+eps); r0 = (1+u)/2; then 3 Newton
                iterations r <- r*(1.5 - 0.5*v*r^2). No ACT tables involved.
                """
                v4 = tp.tile([128, 4], F32, name=f"v4_{tagpfx}", tag="v4")
                for t in range(4):
                    nc.vector.tensor_scalar(v4[:, t:t + 1], mvs[t][:, 1:2],
                                            EPS, None, op0=AL.add)
                u = tp.tile([128, 4], F32, name=f"u_{tagpfx}", tag="u4")
                nc.vector.reciprocal_approx_fast(u[:], v4[:])
                r = tp.tile([128, 4], F32, name=f"r_{tagpfx}", tag="r4")
                nc.vector.tensor_scalar(r[:], u[:], 0.5, 0.5,
                                        op0=AL.mult, op1=AL.add)
                t1 = tp.tile([128, 4], F32, name=f"t1_{tagpfx}", tag="t14")
                t2 = tp.tile([128, 4], F32, name=f"t2_{tagpfx}", tag="t24")
                for _ in range(3):
                    nc.vector.tensor_tensor(t1[:], r[:], r[:], op=AL.mult)
                    nc.vector.tensor_tensor(t2[:], t1[:], v4[:], op=AL.mult)
                    nc.vector.tensor_scalar(t1[:], t2[:], -0.5, 1.5,
                                            op0=AL.mult, op1=AL.add)
                    nc.vector.tensor_tensor(r[:], r[:], t1[:], op=AL.mult)
                return r

            def _transpose4(src_tiles, col, dst, psname):
                """dst[128, 512] f16 = concat_t transpose(src_tiles[t][:, col*128:]).

                4 PE identity-matmul transposes into one PSUM bank, one evac.
                """
                P = ps_tp.tile([128, 4, 128], F32, name=psname, tag="tp")
                for t in range(4):
                    nc.tensor.matmul(P[:, t, :],
                                     src_tiles[t][:, col * 128:(col + 1) * 128],
                                     ident[:], start=True, stop=True)
                nc.vector.tensor_copy(dst[:].rearrange("p (a b) -> p a b", a=4), P[:])

            def phase_a_ln(g):
                st = {}
                x_t = xtiles.pop(g)
                y_t, mv_t = [], []
                for t in range(4):
                    stats = tp.tile([128, 6], F32, name=f"st_{g}_{t}", tag=f"st{t}")
                    nc.vector.bn_stats(stats[:], x_t[t][:])
                    mv = tp.tile([128, 2], F32, name=f"mv_{g}_{t}", tag=f"mv{t}")
                    nc.vector.bn_aggr(mv[:], stats[:])
                    mv_t.append(mv)
                rs4 = _rstd4(mv_t, f"a{g}")
                for t in range(4):
                    yt_ = yp.tile([128, DIM], F16, name=f"y_{g}_{t}", tag=f"y{t}")
                    nc.vector.tensor_scalar(yt_[:], x_t[t][:], mv_t[t][:, 0:1],
                                            rs4[:, t:t + 1],
                                            op0=AL.subtract, op1=AL.mult)
                    y_t.append(yt_)
                    # x is only needed for the proj residual; pre-add projb
                    # here, off phase C's critical path.
                    nc.vector.tensor_tensor(x_t[t][:], x_t[t][:], projb_bc[:],
                                            op=AL.add)
                st["x"] = x_t
                st["y"] = y_t
                return st

            def phase_a_mm(g):
                st = stages[g]
                y_t = st.pop("y")
                yT = []
                for c in range(4):
                    t_ = ytp.tile([128, GRP], F16, name=f"yT_{g}_{c}", tag=f"yT{c}")
                    _transpose4(y_t, c, t_, f"Pt_{g}_{c}")
                    yT.append(t_)
                # Q,K feature-major [128 feat, 512 tok], 2 heads per chunk
                q_t, k_t = [], []
                for m in range(8):
                    P = ps_g.tile([128, GRP], F32, name=f"Pqk_{g}_{m}", tag="gemm")
                    for c in range(4):
                        nc.tensor.matmul(P[:], qkvw_sb[c][:, m * 128:(m + 1) * 128],
                                         yT[c][:], start=(c == 0), stop=(c == 3))
                    pool = qp if m < 4 else kp
                    nm = f"q_{g}_{m}" if m < 4 else f"k_{g}_{m-4}"
                    tg = f"q{m}" if m < 4 else f"k{m-4}"
                    sb = pool.tile([128, GRP], F16, name=nm, tag=tg)
                    nc.scalar.activation(sb[:], P[:], AF.Identity,
                                         bias=qkb_sb[:, m:m + 1])
                    (q_t if m < 4 else k_t).append(sb)
                st["q"], st["k"] = q_t, k_t
                # V token-major with ones column: [128 tok, 8, 65]
                v_t = []
                for t in range(4):
                    P = ps_g.tile([128, DIM], F32, name=f"Pv_{g}_{t}", tag="gemm")
                    for c in range(4):
                        nc.tensor.matmul(P[:], yT[c][:, t * 128:(t + 1) * 128],
                                         qkvw_sb[c][:, 2 * DIM:3 * DIM],
                                         start=(c == 0), stop=(c == 3))
                    vt = vp.tile([128, HEADS, HD + 1], F16, name=f"v_{g}_{t}", tag=f"v{t}")
                    nc.vector.memset(vt[:, :, HD:HD + 1], 1.0)
                    nc.vector.tensor_tensor(
                        vt[:, :, 0:HD],
                        P[:].rearrange("p (h d) -> p h d", h=HEADS),
                        vb_bc[:].rearrange("p (h d) -> p h d", h=HEADS),
                        op=AL.add)
                    v_t.append(vt)
                st["v"] = v_t
                return st

            def b_head(g, h):
                cur = stages[g]
                if h == 0:
                    cur["attn"] = [ap_.tile([128, GRP], F16, name=f"attn_{g}_{c}",
                                            tag=f"attn{c}") for c in range(4)]
                    cur["rrec"] = [rp.tile([1, 2, GRP], F32, name=f"rr_{g}_{c}",
                                           tag=f"rr{c}") for c in range(4)]
                    cur["srow"] = [rp.tile([1, 2, GRP], F32, name=f"sr_{g}_{c}",
                                           tag=f"sr{c}") for c in range(4)]
                attn_sb, srow = cur["attn"], cur["srow"]
                c, half = h // 2, (h % 2) * 64
                P_av = ps_a.tile([65, GRP], F32, name=f"Pav_{g}_{h}", tag="av")
                for wi in range(4):
                    w = 4 * g + wi
                    js = [j for j in (w - 1, w, w + 1) if 0 <= j < n_blocks]
                    P_sim = ps_s.tile([128, 3, 128], F32, name=f"Ps_{g}_{h}_{wi}",
                                      tag="sim")
                    for j in js:
                        d = j - w + 1
                        gj, s = divmod(j, 4)
                        st = stages[gj]
                        nc.tensor.matmul(
                            P_sim[:, d, :],
                            st["k"][c][half:half + 64, s * 128:(s + 1) * 128],
                            cur["q"][c][half:half + 64, wi * 128:(wi + 1) * 128],
                            start=True, stop=True)
                    dlo, dhi = js[0] - w + 1, js[-1] - w + 1
                    E = ep.tile([128, 3, 128], F16, name=f"E_{g}_{h}_{wi}", tag="E")
                    nc.scalar.activation(E[:, dlo:dhi + 1, :], P_sim[:, dlo:dhi + 1, :],
                                         AF.Exp, scale=float(HD) ** -0.5)
                    for ji, j in enumerate(js):
                        d = j - w + 1
                        gj, s = divmod(j, 4)
                        st = stages[gj]
                        nc.tensor.matmul(
                            P_av[:, wi * 128:(wi + 1) * 128],
                            st["v"][s][:, h, :],
                            E[:, d, :],
                            start=(ji == 0), stop=(ji == len(js) - 1))
                # unnormalized out rows + stage the sums row for reciprocal
                nc.scalar.activation(attn_sb[c][half:half + 64, :], P_av[0:64, :],
                                     AF.Copy)
                nc.scalar.activation(srow[c][:, h % 2, :], P_av[64:65, :], AF.Copy)

            def b_norm(g):
                cur = stages[g]
                attn_sb, rrec, srow = cur["attn"], cur.pop("rrec"), cur.pop("srow")
                # partition-broadcast of 1/sum via K=1 mask matmuls, then normalize
                for c in range(4):
                    nc.vector.reciprocal_approx_fast(rrec[c][:], srow[c][:])
                    P_rbc = ps_g.tile([128, GRP], F32, name=f"Prb_{g}_{c}", tag="gemm")
                    nc.tensor.matmul(P_rbc[:], mask_lo[:], rrec[c][:, 0, :],
                                     start=True, stop=False)
                    nc.tensor.matmul(P_rbc[:], mask_hi[:], rrec[c][:, 1, :],
                                     start=False, stop=True)
                    nc.vector.tensor_tensor(attn_sb[c][:], attn_sb[c][:], P_rbc[:],
                                            op=AL.mult)

            def phase_c1(g):
                cur = stages[g]
                attn_sb = cur["attn"]
                # proj + residual -> x1 (token-major; projb pre-added into x)
                x1_t, mv2_t = [], []
                for t in range(4):
                    P = ps_g.tile([128, DIM], F32, name=f"Ppr_{g}_{t}", tag="gemm")
                    for c in range(4):
                        nc.tensor.matmul(P[:], attn_sb[c][:, t * 128:(t + 1) * 128],
                                         projw_sb[c][:], start=(c == 0), stop=(c == 3))
                    x1 = x1p.tile([128, DIM], F32, name=f"x1_{g}_{t}", tag=f"x1{t}")
                    nc.vector.tensor_tensor(x1[:], P[:], cur["x"][t][:], op=AL.add)
                    x1_t.append(x1)
                    stats = tp.tile([128, 6], F32, name=f"st2_{g}_{t}", tag=f"st2{t}")
                    nc.vector.bn_stats(stats[:], x1[:])
                    mv = tp.tile([128, 2], F32, name=f"mv2_{g}_{t}", tag=f"mv2{t}")
                    nc.vector.bn_aggr(mv[:], stats[:])
                    mv2_t.append(mv)
                rs4 = _rstd4(mv2_t, f"c{g}")
                # LN2 -> h2 fp16 token-major
                h2_t = []
                for t in range(4):
                    h2 = h2p.tile([128, DIM], F16, name=f"h2_{g}_{t}", tag=f"h2{t}")
                    nc.vector.tensor_scalar(h2[:], x1_t[t][:], mv2_t[t][:, 0:1],
                                            rs4[:, t:t + 1],
                                            op0=AL.subtract, op1=AL.mult)
                    h2_t.append(h2)
                    # x1 is only needed for the mlp residual now; pre-add b2
                    # off the mlp2 critical path.
                    nc.vector.tensor_tensor(x1_t[t][:], x1_t[t][:], b2_bc[:],
                                            op=AL.add)
                cur["x1"], cur["h2"] = x1_t, h2_t
                del cur["x"], cur["attn"]

            def phase_c2a(g):
                cur = stages[g]
                h2_t = cur.pop("h2")
                h2T = []
                for c in range(4):
                    t_ = h2tp.tile([128, GRP], F16, name=f"h2T_{g}_{c}", tag=f"h2T{c}")
                    _transpose4(h2_t, c, t_, f"Pt2_{g}_{c}")
                    h2T.append(t_)
                # MLP1 + gelu (feature-major)
                gel = []
                for f in range(16):
                    P = ps_g.tile([128, GRP], F32, name=f"Pm1_{g}_{f}", tag="gemm")
                    for c in range(4):
                        nc.tensor.matmul(P[:], w1_sb[c][:, f * 128:(f + 1) * 128],
                                         h2T[c][:], start=(c == 0), stop=(c == 3))
                    gl = gp.tile([128, GRP], F16, name=f"gel_{g}_{f}", tag=f"gel{f}")
                    nc.scalar.activation(gl[:], P[:], AF.Gelu, bias=b1_sb[:, f:f + 1])
                    gel.append(gl)
                cur["gel"] = gel

            def mlp2_chunk(g, t):
                # MLP2 + residual -> out (token-major; b2 pre-added into x1)
                cur = stages[g]
                gel, x1_t = cur["gel"], cur["x1"]
                P = ps_g.tile([128, DIM], F32, name=f"Pm2_{g}_{t}", tag="gemm")
                for f in range(16):
                    nc.tensor.matmul(P[:], gel[f][:, t * 128:(t + 1) * 128],
                                     w2_sb[f][:], start=(f == 0), stop=(f == 15))
                ot = op.tile([128, DIM], F32, name=f"o_{g}_{t}", tag=f"o{t}")
                nc.vector.tensor_tensor(ot[:], P[:], x1_t[t][:], op=AL.add)
                nc.sync.dma_start(out_d[(g * 4 + t) * 128:(g * 4 + t + 1) * 128, :],
                                  ot[:])
                if t == 3:
                    del cur["gel"], cur["x1"]

            for s in range(n_groups + 2):
                if s < n_groups:
                    stages[s] = phase_a_ln(s)
                if 2 <= s:
                    phase_c1(s - 2)
                if s + 1 < n_groups:
                    load_x(s + 1)
                if 2 <= s:
                    phase_c2a(s - 2)
                if s < n_groups:
                    phase_a_mm(s)
                # attention for group s-1, interleaved with the previous
                # group's MLP2 big matmuls to keep the PE array HAM-warm
                if 1 <= s <= n_groups:
                    for p in range(4):
                        b_head(s - 1, 2 * p)
                        b_head(s - 1, 2 * p + 1)
                        if 2 <= s:
                            mlp2_chunk(s - 2, p)
                    b_norm(s - 1)
                elif s == n_groups + 1:
                    for p in range(4):
                        mlp2_chunk(s - 2, p)

    nc.compile()
    return nc


_cache = {}


def _get_nc(n_tok):
    if n_tok not in _cache:
        _cache[n_tok] = build(n_tok)
    return _cache[n_tok]


def _prep_in_maps(inputs):
    return _prep(**inputs)


def _prep(x, t_emb, ln1_g, ln1_b, qkv_w, qkv_b, proj_w, proj_b,
          ln2_g, ln2_b, mlp_w1, mlp_b1, mlp_w2, mlp_b2, time_w, time_b):
    x = np.asarray(x, dtype=np.float32)
    t_emb = np.asarray(t_emb, np.float32)
    # host: modulation rows (tiny); fold ln1 gamma/beta + modulation into qkv W/b
    s = t_emb / (1.0 + np.exp(-t_emb))           # silu
    ss = s @ np.asarray(time_w, np.float32) + np.asarray(time_b, np.float32)
    scale, shift = ss[:, :DIM], ss[:, DIM:]
    g1 = np.asarray(ln1_g, np.float32)
    be1 = np.asarray(ln1_b, np.float32)
    arow = g1[None, :] * (1.0 + scale)                      # [B, 512]
    crow = be1[None, :] * (1.0 + scale) + shift             # [B, 512]
    qkv_w = np.asarray(qkv_w, np.float32)
    qkv_b = np.asarray(qkv_b, np.float32)
    # y = xh*arow + crow ; qkv = y@W + b = xh@(diag(arow)W) + (crow@W + b)
    qkvw_eff = arow[:, :, None] * qkv_w[None, :, :]         # [B, 512, 1536]
    qkvb_eff = crow @ qkv_w + qkv_b[None, :]                # [B, 1536]
    # fold ln2 gamma/beta into mlp_w1/b1
    g2 = np.asarray(ln2_g, np.float32)
    be2 = np.asarray(ln2_b, np.float32)
    w1f = np.asarray(mlp_w1, np.float32) * g2[:, None]
    b1f = be2 @ np.asarray(mlp_w1, np.float32) + np.asarray(mlp_b1, np.float32)

    projw16 = np.asarray(proj_w, np.float32).astype(np.float16)
    w116 = w1f.astype(np.float16)
    w216 = np.asarray(mlp_w2, np.float32).astype(np.float16)
    projb = np.asarray(proj_b, np.float32)
    b2 = np.asarray(mlp_b2, np.float32)

    in_maps = []
    nb = x.shape[0]
    for b in range(nb):
        in_maps.append({
            "x": np.ascontiguousarray(x[b]),
            "qkvw": np.ascontiguousarray(qkvw_eff[b].astype(np.float16)),
            "qkvb": np.ascontiguousarray(qkvb_eff[b]),
            "projw": projw16, "projb": projb,
            "w1": w116, "b1": b1f, "w2": w216, "b2": b2,
        })
    return in_maps


def kernel(**inputs):
    in_maps = _prep_in_maps(inputs)
    n_tok = in_maps[0]["x"].shape[0]
    nc = _get_nc(n_tok)
    nb = len(in_maps)
    res = bass_utils.run_bass_kernel_spmd(nc, in_maps, core_ids=list(range(nb)))
    out = np.stack([res.results[b]["out"] for b in range(nb)], axis=0)
    return out


# revision 42
# speedup vs baseline: 1.0094x; 1.0094x over previous
"""Trainium2 Bass kernel for a local-attention transformer block.

Computes, per batch element (one NeuronCore each, 8 cores):
  ss = silu(t_emb) @ time_w + time_b ;  scale, shift = split(ss)
  y  = LN(x) * g1*(1+scale) + (b1*(1+scale)+shift)   (folded host-side into qkv W/b)
  q,k,v = y @ qkv_w + qkv_b  (heads=8, d=64)
  attn: each 128-token window attends to [prev|cur|next] windows
  x1 = x + attn @ proj_w + proj_b
  out = x1 + gelu(LN2(x1) @ w1 + b1') @ w2 + b2       (ln2 g/b host-folded into w1/b1)

Key layout/engine choices (v2):
  - 3-phase software pipeline per 512-token group g: A(g)=LN1+QKV,
    B(g-1)=attention, C(g-2)=proj+MLP; keeps the PE continuously fed.
  - Transposes (y, h2 -> feature-major) via PE matmul against an fp16
    identity (4 per PSUM bank, single evac copy), not DMA xbar.
  - LN rstd = exp(-0.5*ln(var+eps)) so ACT alternates between only the
    natural_log_exp and gelu table sets (2 loads/step instead of ~7).
  - Softmax denominators via ones-column folded into v (row 64 of the AV
    PSUM); normalization broadcast across partitions via a K=2 mask
    matmul instead of DRAM round trips.
  - All matmuls fp16 inputs / fp32 PSUM accumulation.
"""

import numpy as np
from contextlib import ExitStack

import concourse.bass as bass
import concourse.tile as tile
from concourse import bacc, mybir, masks
from concourse import bass_utils

F32 = mybir.dt.float32
F16 = mybir.dt.float16
AF = mybir.ActivationFunctionType
AL = mybir.AluOpType

DIM = 512
HEADS = 8
HD = 64
FF = 2048
WIN = 128
B = 8
NTOK = 8192
EPS = 1e-5
GRP = 512  # tokens per group (4 windows)


def _bcast_row(dram_ap, offset, n):
    """AP reading dram vector [n] broadcast across 128 partitions."""
    return bass.AP(tensor=dram_ap.tensor, offset=offset, ap=[[0, 128], [1, n]])


def _col_view(dram_ap, offset, ncol):
    """AP reading dram vector [128*ncol] as [128, ncol] feature-major columns."""
    return bass.AP(tensor=dram_ap.tensor, offset=offset, ap=[[1, 128], [128, ncol]])


def build(n_tok=NTOK):
    n_groups = n_tok // GRP
    n_blocks = n_tok // WIN
    nc = bacc.Bacc("TRN2", target_bir_lowering=False, debug=False)

    x_d = nc.dram_tensor("x", [n_tok, DIM], F32, kind="ExternalInput")
    qkvw_d = nc.dram_tensor("qkvw", [DIM, 3 * DIM], F16, kind="ExternalInput")
    qkvb_d = nc.dram_tensor("qkvb", [3 * DIM], F32, kind="ExternalInput")
    projw_d = nc.dram_tensor("projw", [DIM, DIM], F16, kind="ExternalInput")
    projb_d = nc.dram_tensor("projb", [DIM], F32, kind="ExternalInput")
    w1_d = nc.dram_tensor("w1", [DIM, FF], F16, kind="ExternalInput")
    b1_d = nc.dram_tensor("b1", [FF], F32, kind="ExternalInput")
    w2_d = nc.dram_tensor("w2", [FF, DIM], F16, kind="ExternalInput")
    b2_d = nc.dram_tensor("b2", [DIM], F32, kind="ExternalInput")
    out_d = nc.dram_tensor("out", [n_tok, DIM], F32, kind="ExternalOutput")

    with tile.TileContext(nc) as tc:
        with ExitStack() as ctx:
            consts = ctx.enter_context(tc.tile_pool(name="consts", bufs=1))
            xp = ctx.enter_context(tc.tile_pool(name="xp", bufs=3))
            tp = ctx.enter_context(tc.tile_pool(name="tp", bufs=2))
            yp = ctx.enter_context(tc.tile_pool(name="yp", bufs=1))
            ytp = ctx.enter_context(tc.tile_pool(name="ytp", bufs=1))
            qp = ctx.enter_context(tc.tile_pool(name="qp", bufs=2))
            kp = ctx.enter_context(tc.tile_pool(name="kp", bufs=3))
            vp = ctx.enter_context(tc.tile_pool(name="vp", bufs=3))
            ep = ctx.enter_context(tc.tile_pool(name="ep", bufs=4))
            ap_ = ctx.enter_context(tc.tile_pool(name="ap", bufs=2))
            rp = ctx.enter_context(tc.tile_pool(name="rp", bufs=1))
            x1p = ctx.enter_context(tc.tile_pool(name="x1p", bufs=1))
            h2p = ctx.enter_context(tc.tile_pool(name="h2p", bufs=1))
            h2tp = ctx.enter_context(tc.tile_pool(name="h2tp", bufs=1))
            gp = ctx.enter_context(tc.tile_pool(name="gp", bufs=1))
            op = ctx.enter_context(tc.tile_pool(name="op", bufs=1))
            ps_g = ctx.enter_context(tc.tile_pool(name="ps_g", bufs=2, space="PSUM"))
            ps_tp = ctx.enter_context(tc.tile_pool(name="ps_tp", bufs=1, space="PSUM"))
            ps_s = ctx.enter_context(tc.tile_pool(name="ps_s", bufs=3, space="PSUM"))
            ps_a = ctx.enter_context(tc.tile_pool(name="ps_a", bufs=2, space="PSUM"))

            # ---- x prefetch (group 0 first, ahead of the weight DMAs) ----
            xtiles = {}

            def load_x(g):
                ts_ = []
                for t in range(4):
                    xt = xp.tile([128, DIM], F32, name=f"x_{g}_{t}", tag=f"x{t}")
                    nc.sync.dma_start(xt[:],
                                      x_d[(g * 4 + t) * 128:(g * 4 + t + 1) * 128, :])
                    ts_.append(xt)
                xtiles[g] = ts_

            load_x(0)

            # ---- constants ----
            qkvw_sb = []
            for c in range(4):
                t = consts.tile([128, 3 * DIM], F16, name=f"qkvw{c}", tag=f"qkvw{c}")
                nc.sync.dma_start(t[:], qkvw_d[c * 128:(c + 1) * 128, :])
                qkvw_sb.append(t)
            projw_sb = []
            for c in range(4):
                t = consts.tile([128, DIM], F16, name=f"projw{c}", tag=f"projw{c}")
                nc.sync.dma_start(t[:], projw_d[c * 128:(c + 1) * 128, :])
                projw_sb.append(t)
            w1_sb = []
            for c in range(4):
                t = consts.tile([128, FF], F16, name=f"w1_{c}", tag=f"w1_{c}")
                nc.sync.dma_start(t[:], w1_d[c * 128:(c + 1) * 128, :])
                w1_sb.append(t)
            w2_sb = []
            for f in range(16):
                t = consts.tile([128, DIM], F16, name=f"w2_{f}", tag=f"w2_{f}")
                nc.sync.dma_start(t[:], w2_d[f * 128:(f + 1) * 128, :])
                w2_sb.append(t)

            projb_bc = consts.tile([128, DIM], F32, name="projb_bc")
            nc.sync.dma_start(projb_bc[:], _bcast_row(projb_d.ap(), 0, DIM))
            b2_bc = consts.tile([128, DIM], F32, name="b2_bc")
            nc.sync.dma_start(b2_bc[:], _bcast_row(b2_d.ap(), 0, DIM))
            vb_bc = consts.tile([128, DIM], F32, name="vb_bc")
            nc.sync.dma_start(vb_bc[:], _bcast_row(qkvb_d.ap(), 2 * DIM, DIM))
            qkb_sb = consts.tile([128, 8], F32, name="qkb_sb")
            nc.sync.dma_start(qkb_sb[:], _col_view(qkvb_d.ap(), 0, 8))
            b1_sb = consts.tile([128, 16], F32, name="b1_sb")
            nc.sync.dma_start(b1_sb[:], _col_view(b1_d.ap(), 0, 16))
            eps_t = consts.tile([128, 1], F32, name="eps_t")
            nc.vector.memset(eps_t[:], EPS)
            ident = consts.tile([128, 128], F16, name="ident")
            masks.make_identity(nc, ident[:])
            # half-masks: [1,128] fp32 selecting partitions 0-63 / 64-127
            mask_lo = consts.tile([1, 128], F32, name="mask_lo")
            nc.vector.memset(mask_lo[:], 0.0)
            nc.vector.memset(mask_lo[0:1, 0:64], 1.0)
            mask_hi = consts.tile([1, 128], F32, name="mask_hi")
            nc.vector.memset(mask_hi[:], 0.0)
            nc.vector.memset(mask_hi[0:1, 64:128], 1.0)

            stages = {}

            def _rstd4(mvs, tagpfx):
                """rstd[128,4] = 1/sqrt(var+eps) for 4 tiles, DVE-only.

                u = fast_reciprocal(v+eps); r0 = (1+u)/2; then 3 Newton
                iterations r <- r*(1.5 - 0.5*v*r^2). No ACT tables involved.
                """
                v4 = tp.tile([128, 4], F32, name=f"v4_{tagpfx}", tag="v4")
                for t in range(4):
                    nc.vector.tensor_scalar(v4[:, t:t + 1], mvs[t][:, 1:2],
                                            EPS, None, op0=AL.add)
                u = tp.tile([128, 4], F32, name=f"u_{tagpfx}", tag="u4")
                nc.vector.reciprocal_approx_fast(u[:], v4[:])
                r = tp.tile([128, 4], F32, name=f"r_{tagpfx}", tag="r4")
                nc.vector.tensor_scalar(r[:], u[:], 0.5, 0.5,
                                        op0=AL.mult, op1=AL.add)
                t1 = tp.tile([128, 4], F32, name=f"t1_{tagpfx}", tag="t14")
                t2 = tp.tile([128, 4], F32, name=f"t2_{tagpfx}", tag="t24")
                for _ in range(3):
                    nc.vector.tensor_tensor(t1[:], r[:], r[:], op=AL.mult)
                    nc.vector.tensor_tensor(t2[:], t1[:], v4[:], op=AL.mult)
                    nc.vector.tensor_scalar(t1[:], t2[:], -0.5, 1.5,
                                            op0=AL.mult, op1=AL.add)
                    nc.vector.tensor_tensor(r[:], r[:], t1[:], op=AL.mult)
                return r

            def _transpose4(src_tiles, col, dst, psname):
                """dst[128, 512] f16 = concat_t transpose(src_tiles[t][:, col*128:]).

                4 PE identity-matmul transposes into one PSUM bank, one evac.
                """
                P = ps_tp.tile([128, 4, 128], F32, name=psname, tag="tp")
                for t in range(4):
                    nc.tensor.matmul(P[:, t, :],
                                     src_tiles[t][:, col * 128:(col + 1) * 128],
                                     ident[:], start=True, stop=True)
                nc.vector.tensor_copy(dst[:].rearrange("p (a b) -> p a b", a=4), P[:])

            def phase_a_ln(g):
                st = {}
                x_t = xtiles.pop(g)
                y_t, mv_t = [], []
                for t in range(4):
                    stats = tp.tile([128, 6], F32, name=f"st_{g}_{t}", tag=f"st{t}")
                    nc.vector.bn_stats(stats[:], x_t[t][:])
                    mv = tp.tile([128, 2], F32, name=f"mv_{g}_{t}", tag=f"mv{t}")
                    nc.vector.bn_aggr(mv[:], stats[:])
                    mv_t.append(mv)
                rs4 = _rstd4(mv_t, f"a{g}")
                for t in range(4):
                    yt_ = yp.tile([128, DIM], F16, name=f"y_{g}_{t}", tag=f"y{t}")
                    nc.vector.tensor_scalar(yt_[:], x_t[t][:], mv_t[t][:, 0:1],
                                            rs4[:, t:t + 1],
                                            op0=AL.subtract, op1=AL.mult)
                    y_t.append(yt_)
                    # x is only needed for the proj residual; pre-add projb
                    # here, off phase C's critical path.
                    nc.vector.tensor_tensor(x_t[t][:], x_t[t][:], projb_bc[:],
                                            op=AL.add)
                st["x"] = x_t
                st["y"] = y_t
                return st

            def phase_a_mm(g):
                st = stages[g]
                y_t = st.pop("y")
                yT = []
                for c in range(4):
                    t_ = ytp.tile([128, GRP], F16, name=f"yT_{g}_{c}", tag=f"yT{c}")
                    _transpose4(y_t, c, t_, f"Pt_{g}_{c}")
                    yT.append(t_)
                # Q,K feature-major [128 feat, 512 tok], 2 heads per chunk
                q_t, k_t = [], []
                for m in range(8):
                    P = ps_g.tile([128, GRP], F32, name=f"Pqk_{g}_{m}", tag="gemm")
                    for c in range(4):
                        nc.tensor.matmul(P[:], qkvw_sb[c][:, m * 128:(m + 1) * 128],
                                         yT[c][:], start=(c == 0), stop=(c == 3))
                    pool = qp if m < 4 else kp
                    nm = f"q_{g}_{m}" if m < 4 else f"k_{g}_{m-4}"
                    tg = f"q{m}" if m < 4 else f"k{m-4}"
                    sb = pool.tile([128, GRP], F16, name=nm, tag=tg)
                    nc.scalar.activation(sb[:], P[:], AF.Identity,
                                         bias=qkb_sb[:, m:m + 1])
                    (q_t if m < 4 else k_t).append(sb)
                st["q"], st["k"] = q_t, k_t
                # V token-major with ones column: [128 tok, 8, 65]
                v_t = []
                for t in range(4):
                    P = ps_g.tile([128, DIM], F32, name=f"Pv_{g}_{t}", tag="gemm")
                    for c in range(4):
                        nc.tensor.matmul(P[:], yT[c][:, t * 128:(t + 1) * 128],
                                         qkvw_sb[c][:, 2 * DIM:3 * DIM],
                                         start=(c == 0), stop=(c == 3))
                    vt = vp.tile([128, HEADS, HD + 1], F16, name=f"v_{g}_{t}", tag=f"v{t}")
                    nc.vector.memset(vt[:, :, HD:HD + 1], 1.0)
                    nc.vector.tensor_tensor(
                        vt[:, :, 0:HD],
                        P[:].rearrange("p (h d) -> p h d", h=HEADS),
                        vb_bc[:].rearrange("p (h d) -> p h d", h=HEADS),
                        op=AL.add)
                    v_t.append(vt)
                st["v"] = v_t
                return st

            def b_pair(g, c):
                """Attention for heads 2c, 2c+1 — 64-row sim matmuls packed onto
                the two PE array halves via tile_position so they run
                concurrently."""
                cur = stages[g]
                if c == 0:
                    cur["attn"] = [ap_.tile([128, GRP], F16, name=f"attn_{g}_{cc}",
                                            tag=f"attn{cc}") for cc in range(4)]
                    cur["rrec"] = [rp.tile([1, 2, GRP], F32, name=f"rr_{g}_{cc}",
                                           tag=f"rr{cc}") for cc in range(4)]
                    cur["srow"] = [rp.tile([1, 2, GRP], F32, name=f"sr_{g}_{cc}",
                                           tag=f"sr{cc}") for cc in range(4)]
                attn_sb, srow = cur["attn"], cur["srow"]
                P_av = [ps_a.tile([65, GRP], F32, name=f"Pav_{g}_{c}_{hh}", tag="av")
                        for hh in range(2)]
                for wi in range(4):
                    w = 4 * g + wi
                    js = [j for j in (w - 1, w, w + 1) if 0 <= j < n_blocks]
                    P_sim = [ps_s.tile([128, 3, 128], F32, name=f"Ps_{g}_{c}_{wi}_{hh}",
                                       tag="sim") for hh in range(2)]
                    for j in js:
                        d = j - w + 1
                        gj, s = divmod(j, 4)
                        st = stages[gj]
                        for hh, half in ((0, 0), (1, 64)):
                            nc.tensor.matmul(
                                P_sim[hh][:, d, :],
                                st["k"][c][half:half + 64, s * 128:(s + 1) * 128],
                                cur["q"][c][half:half + 64, wi * 128:(wi + 1) * 128],
                                start=True, stop=True, tile_position=(half, 0))
                    dlo, dhi = js[0] - w + 1, js[-1] - w + 1
                    E = [ep.tile([128, 3, 128], F16, name=f"E_{g}_{c}_{wi}_{hh}",
                                 tag="E") for hh in range(2)]
                    for hh in range(2):
                        nc.scalar.activation(E[hh][:, dlo:dhi + 1, :],
                                             P_sim[hh][:, dlo:dhi + 1, :],
                                             AF.Exp, scale=float(HD) ** -0.5)
                    for ji, j in enumerate(js):
                        d = j - w + 1
                        gj, s = divmod(j, 4)
                        st = stages[gj]
                        for hh in range(2):
                            nc.tensor.matmul(
                                P_av[hh][:, wi * 128:(wi + 1) * 128],
                                st["v"][s][:, 2 * c + hh, :],
                                E[hh][:, d, :],
                                start=(ji == 0), stop=(ji == len(js) - 1))
                # unnormalized out rows + stage the sums rows for reciprocal
                for hh, half in ((0, 0), (1, 64)):
                    nc.scalar.activation(attn_sb[c][half:half + 64, :],
                                         P_av[hh][0:64, :], AF.Copy)
                    nc.scalar.activation(srow[c][:, hh, :], P_av[hh][64:65, :],
                                         AF.Copy)

            def b_norm(g):
                cur = stages[g]
                attn_sb, rrec, srow = cur["attn"], cur.pop("rrec"), cur.pop("srow")
                # partition-broadcast of 1/sum via K=1 mask matmuls, then normalize
                for c in range(4):
                    nc.vector.reciprocal_approx_fast(rrec[c][:], srow[c][:])
                    P_rbc = ps_g.tile([128, GRP], F32, name=f"Prb_{g}_{c}", tag="gemm")
                    nc.tensor.matmul(P_rbc[:], mask_lo[:], rrec[c][:, 0, :],
                                     start=True, stop=False)
                    nc.tensor.matmul(P_rbc[:], mask_hi[:], rrec[c][:, 1, :],
                                     start=False, stop=True)
                    nc.vector.tensor_tensor(attn_sb[c][:], attn_sb[c][:], P_rbc[:],
                                            op=AL.mult)

            def phase_c1(g):
                cur = stages[g]
                attn_sb = cur["attn"]
                # proj + residual -> x1 (token-major; projb pre-added into x)
                x1_t, mv2_t = [], []
                for t in range(4):
                    P = ps_g.tile([128, DIM], F32, name=f"Ppr_{g}_{t}", tag="gemm")
                    for c in range(4):
                        nc.tensor.matmul(P[:], attn_sb[c][:, t * 128:(t + 1) * 128],
                                         projw_sb[c][:], start=(c == 0), stop=(c == 3))
                    x1 = x1p.tile([128, DIM], F32, name=f"x1_{g}_{t}", tag=f"x1{t}")
                    nc.vector.tensor_tensor(x1[:], P[:], cur["x"][t][:], op=AL.add)
                    x1_t.append(x1)
                    stats = tp.tile([128, 6], F32, name=f"st2_{g}_{t}", tag=f"st2{t}")
                    nc.vector.bn_stats(stats[:], x1[:])
                    mv = tp.tile([128, 2], F32, name=f"mv2_{g}_{t}", tag=f"mv2{t}")
                    nc.vector.bn_aggr(mv[:], stats[:])
                    mv2_t.append(mv)
                rs4 = _rstd4(mv2_t, f"c{g}")
                # LN2 -> h2 fp16 token-major
                h2_t = []
                for t in range(4):
                    h2 = h2p.tile([128, DIM], F16, name=f"h2_{g}_{t}", tag=f"h2{t}")
                    nc.vector.tensor_scalar(h2[:], x1_t[t][:], mv2_t[t][:, 0:1],
                                            rs4[:, t:t + 1],
                                            op0=AL.subtract, op1=AL.mult)
                    h2_t.append(h2)
                    # x1 is only needed for the mlp residual now; pre-add b2
                    # off the mlp2 critical path.
                    nc.vector.tensor_tensor(x1_t[t][:], x1_t[t][:], b2_bc[:],
                                            op=AL.add)
                cur["x1"], cur["h2"] = x1_t, h2_t
                del cur["x"], cur["attn"]

            def phase_c2a(g):
                cur = stages[g]
                h2_t = cur.pop("h2")
                h2T = []
                for c in range(4):
                    t_ = h2tp.tile([128, GRP], F16, name=f"h2T_{g}_{c}", tag=f"h2T{c}")
                    _transpose4(h2_t, c, t_, f"Pt2_{g}_{c}")
                    h2T.append(t_)
                # MLP1 + gelu (feature-major)
                gel = []
                for f in range(16):
                    P = ps_g.tile([128, GRP], F32, name=f"Pm1_{g}_{f}", tag="gemm")
                    for c in range(4):
                        nc.tensor.matmul(P[:], w1_sb[c][:, f * 128:(f + 1) * 128],
                                         h2T[c][:], start=(c == 0), stop=(c == 3))
                    gl = gp.tile([128, GRP], F16, name=f"gel_{g}_{f}", tag=f"gel{f}")
                    nc.scalar.activation(gl[:], P[:], AF.Gelu, bias=b1_sb[:, f:f + 1])
                    gel.append(gl)
                cur["gel"] = gel

            def mlp2_chunk(g, t):
                # MLP2 + residual -> out (token-major; b2 pre-added into x1)
                cur = stages[g]
                gel, x1_t = cur["gel"], cur["x1"]
                P = ps_g.tile([128, DIM], F32, name=f"Pm2_{g}_{t}", tag="gemm")
                for f in range(16):
                    nc.tensor.matmul(P[:], gel[f][:, t * 128:(t + 1) * 128],
                                     w2_sb[f][:], start=(f == 0), stop=(f == 15))
                ot = op.tile([128, DIM], F32, name=f"o_{g}_{t}", tag=f"o{t}")
                nc.vector.tensor_tensor(ot[:], P[:], x1_t[t][:], op=AL.add)
                nc.sync.dma_start(out_d[(g * 4 + t) * 128:(g * 4 + t + 1) * 128, :],
                                  ot[:])
                if t == 3:
                    del cur["gel"], cur["x1"]

            for s in range(n_groups + 2):
                if s < n_groups:
                    stages[s] = phase_a_ln(s)
                if 2 <= s:
                    phase_c1(s - 2)
                if s + 1 < n_groups:
                    load_x(s + 1)
                if s < n_groups:
                    phase_a_mm(s)
                if 2 <= s:
                    phase_c2a(s - 2)
                # attention for group s-1, interleaved with the previous
                # group's MLP2 big matmuls to keep the PE array HAM-warm
                if 1 <= s <= n_groups:
                    for p in range(4):
                        b_pair(s - 1, p)
                        if 2 <= s:
                            mlp2_chunk(s - 2, p)
                    b_norm(s - 1)
                elif s == n_groups + 1:
                    for p in range(4):
                        mlp2_chunk(s - 2, p)

    nc.compile()
    return nc


_cache = {}


def _get_nc(n_tok):
    if n_tok not in _cache:
        _cache[n_tok] = build(n_tok)
    return _cache[n_tok]


def _prep_in_maps(inputs):
    return _prep(**inputs)


def _prep(x, t_emb, ln1_g, ln1_b, qkv_w, qkv_b, proj_w, proj_b,
          ln2_g, ln2_b, mlp_w1, mlp_b1, mlp_w2, mlp_b2, time_w, time_b):
    x = np.asarray(x, dtype=np.float32)
    t_emb = np.asarray(t_emb, np.float32)
    # host: modulation rows (tiny); fold ln1 gamma/beta + modulation into qkv W/b
    s = t_emb / (1.0 + np.exp(-t_emb))           # silu
    ss = s @ np.asarray(time_w, np.float32) + np.asarray(time_b, np.float32)
    scale, shift = ss[:, :DIM], ss[:, DIM:]
    g1 = np.asarray(ln1_g, np.float32)
    be1 = np.asarray(ln1_b, np.float32)
    arow = g1[None, :] * (1.0 + scale)                      # [B, 512]
    crow = be1[None, :] * (1.0 + scale) + shift             # [B, 512]
    qkv_w = np.asarray(qkv_w, np.float32)
    qkv_b = np.asarray(qkv_b, np.float32)
    # y = xh*arow + crow ; qkv = y@W + b = xh@(diag(arow)W) + (crow@W + b)
    qkvw_eff = arow[:, :, None] * qkv_w[None, :, :]         # [B, 512, 1536]
    qkvb_eff = crow @ qkv_w + qkv_b[None, :]                # [B, 1536]
    # fold ln2 gamma/beta into mlp_w1/b1
    g2 = np.asarray(ln2_g, np.float32)
    be2 = np.asarray(ln2_b, np.float32)
    w1f = np.asarray(mlp_w1, np.float32) * g2[:, None]
    b1f = be2 @ np.asarray(mlp_w1, np.float32) + np.asarray(mlp_b1, np.float32)

    projw16 = np.asarray(proj_w, np.float32).astype(np.float16)
    w116 = w1f.astype(np.float16)
    w216 = np.asarray(mlp_w2, np.float32).astype(np.float16)
    projb = np.asarray(proj_b, np.float32)
    b2 = np.asarray(mlp_b2, np.float32)

    in_maps = []
    nb = x.shape[0]
    for b in range(nb):
        in_maps.append({
            "x": np.ascontiguousarray(x[b]),
            "qkvw": np.ascontiguousarray(qkvw_eff[b].astype(np.float16)),
            "qkvb": np.ascontiguousarray(qkvb_eff[b]),
            "projw": projw16, "projb": projb,
            "w1": w116, "b1": b1f, "w2": w216, "b2": b2,
        })
    return in_maps


def kernel(**inputs):
    in_maps = _prep_in_maps(inputs)
    n_tok = in_maps[0]["x"].shape[0]
    nc = _get_nc(n_tok)
    nb = len(in_maps)
    res = bass_utils.run_bass_kernel_spmd(nc, in_maps, core_ids=list(range(nb)))
    out = np.stack([res.results[b]["out"] for b in range(nb)], axis=0)
    return out


# revision 45
# speedup vs baseline: 1.0492x; 1.0394x over previous
"""Trainium2 Bass kernel for a local-attention transformer block.

Computes, per batch element (one NeuronCore each, 8 cores):
  ss = silu(t_emb) @ time_w + time_b ;  scale, shift = split(ss)
  y  = LN(x) * g1*(1+scale) + (b1*(1+scale)+shift)   (folded host-side into qkv W/b)
  q,k,v = y @ qkv_w + qkv_b  (heads=8, d=64)
  attn: each 128-token window attends to [prev|cur|next] windows
  x1 = x + attn @ proj_w + proj_b
  out = x1 + gelu(LN2(x1) @ w1 + b1') @ w2 + b2       (ln2 g/b host-folded into w1/b1)

Key layout/engine choices (v2):
  - 3-phase software pipeline per 512-token group g: A(g)=LN1+QKV,
    B(g-1)=attention, C(g-2)=proj+MLP; keeps the PE continuously fed.
  - Transposes (y, h2 -> feature-major) via PE matmul against an fp16
    identity (4 per PSUM bank, single evac copy), not DMA xbar.
  - LN rstd = exp(-0.5*ln(var+eps)) so ACT alternates between only the
    natural_log_exp and gelu table sets (2 loads/step instead of ~7).
  - Softmax denominators via ones-column folded into v (row 64 of the AV
    PSUM); normalization broadcast across partitions via a K=2 mask
    matmul instead of DRAM round trips.
  - All matmuls fp16 inputs / fp32 PSUM accumulation.
"""

import numpy as np
from contextlib import ExitStack

import concourse.bass as bass
import concourse.tile as tile
from concourse import bacc, mybir, masks
from concourse import bass_utils

F32 = mybir.dt.float32
F16 = mybir.dt.float16
AF = mybir.ActivationFunctionType
AL = mybir.AluOpType

DIM = 512
HEADS = 8
HD = 64
FF = 2048
WIN = 128
B = 8
NTOK = 8192
EPS = 1e-5
GRP = 512  # tokens per group (4 windows)


def _bcast_row(dram_ap, offset, n):
    """AP reading dram vector [n] broadcast across 128 partitions."""
    return bass.AP(tensor=dram_ap.tensor, offset=offset, ap=[[0, 128], [1, n]])


def _col_view(dram_ap, offset, ncol):
    """AP reading dram vector [128*ncol] as [128, ncol] feature-major columns."""
    return bass.AP(tensor=dram_ap.tensor, offset=offset, ap=[[1, 128], [128, ncol]])


def build(n_tok=NTOK):
    n_groups = n_tok // GRP
    n_blocks = n_tok // WIN
    nc = bacc.Bacc("TRN2", target_bir_lowering=False, debug=False)

    x_d = nc.dram_tensor("x", [n_tok, DIM], F32, kind="ExternalInput")
    qkvw_d = nc.dram_tensor("qkvw", [DIM, 3 * DIM], F16, kind="ExternalInput")
    qkvb_d = nc.dram_tensor("qkvb", [3 * DIM], F32, kind="ExternalInput")
    projw_d = nc.dram_tensor("projw", [DIM, DIM], F16, kind="ExternalInput")
    projb_d = nc.dram_tensor("projb", [DIM], F32, kind="ExternalInput")
    w1_d = nc.dram_tensor("w1", [DIM, FF], F16, kind="ExternalInput")
    b1_d = nc.dram_tensor("b1", [FF], F32, kind="ExternalInput")
    w2_d = nc.dram_tensor("w2", [FF, DIM], F16, kind="ExternalInput")
    b2_d = nc.dram_tensor("b2", [DIM], F32, kind="ExternalInput")
    out_d = nc.dram_tensor("out", [n_tok, DIM], F32, kind="ExternalOutput")

    with tile.TileContext(nc) as tc:
        with ExitStack() as ctx:
            consts = ctx.enter_context(tc.tile_pool(name="consts", bufs=1))
            xp = ctx.enter_context(tc.tile_pool(name="xp", bufs=3))
            tp = ctx.enter_context(tc.tile_pool(name="tp", bufs=2))
            yp = ctx.enter_context(tc.tile_pool(name="yp", bufs=1))
            ytp = ctx.enter_context(tc.tile_pool(name="ytp", bufs=1))
            qp = ctx.enter_context(tc.tile_pool(name="qp", bufs=2))
            kp = ctx.enter_context(tc.tile_pool(name="kp", bufs=3))
            vp = ctx.enter_context(tc.tile_pool(name="vp", bufs=3))
            ep = ctx.enter_context(tc.tile_pool(name="ep", bufs=4))
            ap_ = ctx.enter_context(tc.tile_pool(name="ap", bufs=2))
            rp = ctx.enter_context(tc.tile_pool(name="rp", bufs=1))
            x1p = ctx.enter_context(tc.tile_pool(name="x1p", bufs=1))
            h2p = ctx.enter_context(tc.tile_pool(name="h2p", bufs=1))
            h2tp = ctx.enter_context(tc.tile_pool(name="h2tp", bufs=1))
            gp = ctx.enter_context(tc.tile_pool(name="gp", bufs=1))
            op = ctx.enter_context(tc.tile_pool(name="op", bufs=1))
            ps_g = ctx.enter_context(tc.tile_pool(name="ps_g", bufs=2, space="PSUM"))
            ps_tp = ctx.enter_context(tc.tile_pool(name="ps_tp", bufs=1, space="PSUM"))
            ps_s = ctx.enter_context(tc.tile_pool(name="ps_s", bufs=3, space="PSUM"))
            ps_a = ctx.enter_context(tc.tile_pool(name="ps_a", bufs=2, space="PSUM"))

            # ---- x prefetch (group 0 first, ahead of the weight DMAs) ----
            xtiles = {}

            def load_x(g):
                ts_ = []
                for t in range(4):
                    xt = xp.tile([128, DIM], F32, name=f"x_{g}_{t}", tag=f"x{t}")
                    nc.sync.dma_start(xt[:],
                                      x_d[(g * 4 + t) * 128:(g * 4 + t + 1) * 128, :])
                    ts_.append(xt)
                xtiles[g] = ts_

            load_x(0)

            # ---- constants ----
            qkvw_sb = []
            for c in range(4):
                t = consts.tile([128, 3 * DIM], F16, name=f"qkvw{c}", tag=f"qkvw{c}")
                nc.sync.dma_start(t[:], qkvw_d[c * 128:(c + 1) * 128, :])
                qkvw_sb.append(t)
            projw_sb = []
            for c in range(4):
                t = consts.tile([128, DIM], F16, name=f"projw{c}", tag=f"projw{c}")
                nc.sync.dma_start(t[:], projw_d[c * 128:(c + 1) * 128, :])
                projw_sb.append(t)
            w1_sb = []
            for c in range(4):
                t = consts.tile([128, FF], F16, name=f"w1_{c}", tag=f"w1_{c}")
                nc.sync.dma_start(t[:], w1_d[c * 128:(c + 1) * 128, :])
                w1_sb.append(t)
            w2_sb = []
            for f in range(16):
                t = consts.tile([128, DIM], F16, name=f"w2_{f}", tag=f"w2_{f}")
                nc.sync.dma_start(t[:], w2_d[f * 128:(f + 1) * 128, :])
                w2_sb.append(t)

            projb_bc = consts.tile([128, DIM], F32, name="projb_bc")
            nc.sync.dma_start(projb_bc[:], _bcast_row(projb_d.ap(), 0, DIM))
            b2_bc = consts.tile([128, DIM], F32, name="b2_bc")
            nc.sync.dma_start(b2_bc[:], _bcast_row(b2_d.ap(), 0, DIM))
            vb_bc = consts.tile([128, DIM], F32, name="vb_bc")
            nc.sync.dma_start(vb_bc[:], _bcast_row(qkvb_d.ap(), 2 * DIM, DIM))
            qkb_sb = consts.tile([128, 8], F32, name="qkb_sb")
            nc.sync.dma_start(qkb_sb[:], _col_view(qkvb_d.ap(), 0, 8))
            b1_sb = consts.tile([128, 16], F32, name="b1_sb")
            nc.sync.dma_start(b1_sb[:], _col_view(b1_d.ap(), 0, 16))
            eps_t = consts.tile([128, 1], F32, name="eps_t")
            nc.vector.memset(eps_t[:], EPS)
            ident = consts.tile([128, 128], F16, name="ident")
            masks.make_identity(nc, ident[:])
            # half-masks: [1,128] fp32 selecting partitions 0-63 / 64-127
            mask_lo = consts.tile([1, 128], F32, name="mask_lo")
            nc.vector.memset(mask_lo[:], 0.0)
            nc.vector.memset(mask_lo[0:1, 0:64], 1.0)
            mask_hi = consts.tile([1, 128], F32, name="mask_hi")
            nc.vector.memset(mask_hi[:], 0.0)
            nc.vector.memset(mask_hi[0:1, 64:128], 1.0)

            stages = {}

            def _rstd4(mvs, tagpfx):
                """rstd[128,4] = 1/sqrt(var+eps) for 4 tiles, DVE-only.

                u = fast_reciprocal(v+eps); r0 = (1+u)/2; then 3 Newton
                iterations r <- r*(1.5 - 0.5*v*r^2). No ACT tables involved.
                """
                v4 = tp.tile([128, 4], F32, name=f"v4_{tagpfx}", tag="v4")
                for t in range(4):
                    nc.vector.tensor_scalar(v4[:, t:t + 1], mvs[t][:, 1:2],
                                            EPS, None, op0=AL.add)
                u = tp.tile([128, 4], F32, name=f"u_{tagpfx}", tag="u4")
                nc.vector.reciprocal_approx_fast(u[:], v4[:])
                r = tp.tile([128, 4], F32, name=f"r_{tagpfx}", tag="r4")
                nc.vector.tensor_scalar(r[:], u[:], 0.5, 0.5,
                                        op0=AL.mult, op1=AL.add)
                t1 = tp.tile([128, 4], F32, name=f"t1_{tagpfx}", tag="t14")
                t2 = tp.tile([128, 4], F32, name=f"t2_{tagpfx}", tag="t24")
                for _ in range(3):
                    nc.vector.tensor_tensor(t1[:], r[:], r[:], op=AL.mult)
                    nc.vector.tensor_tensor(t2[:], t1[:], v4[:], op=AL.mult)
                    nc.vector.tensor_scalar(t1[:], t2[:], -0.5, 1.5,
                                            op0=AL.mult, op1=AL.add)
                    nc.vector.tensor_tensor(r[:], r[:], t1[:], op=AL.mult)
                return r

            def _transpose4(src_tiles, col, dst, psname):
                """dst[128, 512] f16 = concat_t transpose(src_tiles[t][:, col*128:]).

                4 PE identity-matmul transposes into one PSUM bank, one evac.
                """
                P = ps_tp.tile([128, 4, 128], F32, name=psname, tag="tp")
                for t in range(4):
                    nc.tensor.matmul(P[:, t, :],
                                     src_tiles[t][:, col * 128:(col + 1) * 128],
                                     ident[:], start=True, stop=True)
                nc.vector.tensor_copy(dst[:].rearrange("p (a b) -> p a b", a=4), P[:])

            def phase_a_ln(g):
                st = {}
                x_t = xtiles.pop(g)
                y_t, mv_t = [], []
                for t in range(4):
                    stats = tp.tile([128, 6], F32, name=f"st_{g}_{t}", tag=f"st{t}")
                    nc.vector.bn_stats(stats[:], x_t[t][:])
                    mv = tp.tile([128, 2], F32, name=f"mv_{g}_{t}", tag=f"mv{t}")
                    nc.vector.bn_aggr(mv[:], stats[:])
                    mv_t.append(mv)
                rs4 = _rstd4(mv_t, f"a{g}")
                for t in range(4):
                    yt_ = yp.tile([128, DIM], F16, name=f"y_{g}_{t}", tag=f"y{t}")
                    nc.vector.tensor_scalar(yt_[:], x_t[t][:], mv_t[t][:, 0:1],
                                            rs4[:, t:t + 1],
                                            op0=AL.subtract, op1=AL.mult)
                    y_t.append(yt_)
                    # x is only needed for the proj residual; pre-add projb
                    # here, off phase C's critical path.
                    nc.vector.tensor_tensor(x_t[t][:], x_t[t][:], projb_bc[:],
                                            op=AL.add)
                st["x"] = x_t
                st["y"] = y_t
                return st

            def phase_a_mm1(g):
                st = stages[g]
                y_t = st.pop("y")
                yT = []
                for c in range(4):
                    t_ = ytp.tile([128, GRP], F16, name=f"yT_{g}_{c}", tag=f"yT{c}")
                    _transpose4(y_t, c, t_, f"Pt_{g}_{c}")
                    yT.append(t_)
                st["yT"] = yT
                # Q,K feature-major [128 feat, 512 tok], 2 heads per chunk
                q_t, k_t = [], []
                for m in range(8):
                    P = ps_g.tile([128, GRP], F32, name=f"Pqk_{g}_{m}", tag="gemm")
                    for c in range(4):
                        nc.tensor.matmul(P[:], qkvw_sb[c][:, m * 128:(m + 1) * 128],
                                         yT[c][:], start=(c == 0), stop=(c == 3))
                    pool = qp if m < 4 else kp
                    nm = f"q_{g}_{m}" if m < 4 else f"k_{g}_{m-4}"
                    tg = f"q{m}" if m < 4 else f"k{m-4}"
                    sb = pool.tile([128, GRP], F16, name=nm, tag=tg)
                    nc.vector.tensor_scalar(sb[:], P[:], qkb_sb[:, m:m + 1], None,
                                            op0=AL.add)
                    (q_t if m < 4 else k_t).append(sb)
                st["q"], st["k"] = q_t, k_t

            def phase_a_mm2(g):
                st = stages[g]
                yT = st.pop("yT")
                # V token-major with ones column: [128 tok, 8, 65]
                v_t = []
                for t in range(4):
                    P = ps_g.tile([128, DIM], F32, name=f"Pv_{g}_{t}", tag="gemm")
                    for c in range(4):
                        nc.tensor.matmul(P[:], yT[c][:, t * 128:(t + 1) * 128],
                                         qkvw_sb[c][:, 2 * DIM:3 * DIM],
                                         start=(c == 0), stop=(c == 3))
                    vt = vp.tile([128, HEADS, HD + 1], F16, name=f"v_{g}_{t}", tag=f"v{t}")
                    nc.vector.memset(vt[:, :, HD:HD + 1], 1.0)
                    nc.vector.tensor_tensor(
                        vt[:, :, 0:HD],
                        P[:].rearrange("p (h d) -> p h d", h=HEADS),
                        vb_bc[:].rearrange("p (h d) -> p h d", h=HEADS),
                        op=AL.add)
                    v_t.append(vt)
                st["v"] = v_t

            def b_pair(g, c):
                """Attention for heads 2c, 2c+1 — 64-row sim matmuls packed onto
                the two PE array halves via tile_position so they run
                concurrently."""
                cur = stages[g]
                if c == 0:
                    cur["attn"] = [ap_.tile([128, GRP], F16, name=f"attn_{g}_{cc}",
                                            tag=f"attn{cc}") for cc in range(4)]
                    cur["rrec"] = [rp.tile([1, 2, GRP], F32, name=f"rr_{g}_{cc}",
                                           tag=f"rr{cc}") for cc in range(4)]
                    cur["srow"] = [rp.tile([1, 2, GRP], F32, name=f"sr_{g}_{cc}",
                                           tag=f"sr{cc}") for cc in range(4)]
                attn_sb, srow = cur["attn"], cur["srow"]
                P_av = [ps_a.tile([65, GRP], F32, name=f"Pav_{g}_{c}_{hh}", tag="av")
                        for hh in range(2)]
                for wi in range(4):
                    w = 4 * g + wi
                    js = [j for j in (w - 1, w, w + 1) if 0 <= j < n_blocks]
                    P_sim = [ps_s.tile([128, 3, 128], F32, name=f"Ps_{g}_{c}_{wi}_{hh}",
                                       tag="sim") for hh in range(2)]
                    for j in js:
                        d = j - w + 1
                        gj, s = divmod(j, 4)
                        st = stages[gj]
                        for hh, half in ((0, 0), (1, 64)):
                            nc.tensor.matmul(
                                P_sim[hh][:, d, :],
                                st["k"][c][half:half + 64, s * 128:(s + 1) * 128],
                                cur["q"][c][half:half + 64, wi * 128:(wi + 1) * 128],
                                start=True, stop=True, tile_position=(half, 0))
                    dlo, dhi = js[0] - w + 1, js[-1] - w + 1
                    E = [ep.tile([128, 3, 128], F16, name=f"E_{g}_{c}_{wi}_{hh}",
                                 tag="E") for hh in range(2)]
                    for hh in range(2):
                        nc.scalar.activation(E[hh][:, dlo:dhi + 1, :],
                                             P_sim[hh][:, dlo:dhi + 1, :],
                                             AF.Exp, scale=float(HD) ** -0.5)
                    for ji, j in enumerate(js):
                        d = j - w + 1
                        gj, s = divmod(j, 4)
                        st = stages[gj]
                        for hh in range(2):
                            nc.tensor.matmul(
                                P_av[hh][:, wi * 128:(wi + 1) * 128],
                                st["v"][s][:, 2 * c + hh, :],
                                E[hh][:, d, :],
                                start=(ji == 0), stop=(ji == len(js) - 1))
                # unnormalized out rows + stage the sums rows for reciprocal
                for hh, half in ((0, 0), (1, 64)):
                    nc.scalar.activation(attn_sb[c][half:half + 64, :],
                                         P_av[hh][0:64, :], AF.Copy)
                    nc.scalar.activation(srow[c][:, hh, :], P_av[hh][64:65, :],
                                         AF.Copy)

            def b_norm(g):
                cur = stages[g]
                attn_sb, rrec, srow = cur["attn"], cur.pop("rrec"), cur.pop("srow")
                # partition-broadcast of 1/sum via K=1 mask matmuls, then normalize
                for c in range(4):
                    nc.vector.reciprocal_approx_fast(rrec[c][:], srow[c][:])
                    P_rbc = ps_g.tile([128, GRP], F32, name=f"Prb_{g}_{c}", tag="gemm")
                    nc.tensor.matmul(P_rbc[:], mask_lo[:], rrec[c][:, 0, :],
                                     start=True, stop=False)
                    nc.tensor.matmul(P_rbc[:], mask_hi[:], rrec[c][:, 1, :],
                                     start=False, stop=True)
                    nc.vector.tensor_tensor(attn_sb[c][:], attn_sb[c][:], P_rbc[:],
                                            op=AL.mult)

            def phase_c1(g):
                cur = stages[g]
                attn_sb = cur["attn"]
                # proj + residual -> x1 (token-major; projb pre-added into x)
                x1_t, mv2_t = [], []
                for t in range(4):
                    P = ps_g.tile([128, DIM], F32, name=f"Ppr_{g}_{t}", tag="gemm")
                    for c in range(4):
                        nc.tensor.matmul(P[:], attn_sb[c][:, t * 128:(t + 1) * 128],
                                         projw_sb[c][:], start=(c == 0), stop=(c == 3))
                    x1 = x1p.tile([128, DIM], F32, name=f"x1_{g}_{t}", tag=f"x1{t}")
                    nc.vector.tensor_tensor(x1[:], P[:], cur["x"][t][:], op=AL.add)
                    x1_t.append(x1)
                    stats = tp.tile([128, 6], F32, name=f"st2_{g}_{t}", tag=f"st2{t}")
                    nc.vector.bn_stats(stats[:], x1[:])
                    mv = tp.tile([128, 2], F32, name=f"mv2_{g}_{t}", tag=f"mv2{t}")
                    nc.vector.bn_aggr(mv[:], stats[:])
                    mv2_t.append(mv)
                rs4 = _rstd4(mv2_t, f"c{g}")
                # LN2 -> h2 fp16 token-major
                h2_t = []
                for t in range(4):
                    h2 = h2p.tile([128, DIM], F16, name=f"h2_{g}_{t}", tag=f"h2{t}")
                    nc.vector.tensor_scalar(h2[:], x1_t[t][:], mv2_t[t][:, 0:1],
                                            rs4[:, t:t + 1],
                                            op0=AL.subtract, op1=AL.mult)
                    h2_t.append(h2)
                    # x1 is only needed for the mlp residual now; pre-add b2
                    # off the mlp2 critical path.
                    nc.vector.tensor_tensor(x1_t[t][:], x1_t[t][:], b2_bc[:],
                                            op=AL.add)
                cur["x1"], cur["h2"] = x1_t, h2_t
                del cur["x"], cur["attn"]

            def phase_c2a1(g):
                cur = stages[g]
                h2_t = cur.pop("h2")
                h2T = []
                for c in range(4):
                    t_ = h2tp.tile([128, GRP], F16, name=f"h2T_{g}_{c}", tag=f"h2T{c}")
                    _transpose4(h2_t, c, t_, f"Pt2_{g}_{c}")
                    h2T.append(t_)
                cur["h2T"] = h2T

            def phase_c2a2(g):
                cur = stages[g]
                h2T = cur.pop("h2T")
                # MLP1 + gelu (feature-major)
                gel = []
                for f in range(16):
                    P = ps_g.tile([128, GRP], F32, name=f"Pm1_{g}_{f}", tag="gemm")
                    for c in range(4):
                        nc.tensor.matmul(P[:], w1_sb[c][:, f * 128:(f + 1) * 128],
                                         h2T[c][:], start=(c == 0), stop=(c == 3))
                    gl = gp.tile([128, GRP], F16, name=f"gel_{g}_{f}", tag=f"gel{f}")
                    nc.scalar.activation(gl[:], P[:], AF.Gelu, bias=b1_sb[:, f:f + 1])
                    gel.append(gl)
                cur["gel"] = gel

            def mlp2_chunk(g, t):
                # MLP2 + residual -> out (token-major; b2 pre-added into x1)
                cur = stages[g]
                gel, x1_t = cur["gel"], cur["x1"]
                P = ps_g.tile([128, DIM], F32, name=f"Pm2_{g}_{t}", tag="gemm")
                for f in range(16):
                    nc.tensor.matmul(P[:], gel[f][:, t * 128:(t + 1) * 128],
                                     w2_sb[f][:], start=(f == 0), stop=(f == 15))
                ot = op.tile([128, DIM], F32, name=f"o_{g}_{t}", tag=f"o{t}")
                nc.vector.tensor_tensor(ot[:], P[:], x1_t[t][:], op=AL.add)
                nc.sync.dma_start(out_d[(g * 4 + t) * 128:(g * 4 + t + 1) * 128, :],
                                  ot[:])
                if t == 3:
                    del cur["gel"], cur["x1"]

            for s in range(n_groups + 2):
                if s < n_groups:
                    stages[s] = phase_a_ln(s)
                if 2 <= s:
                    phase_c1(s - 2)
                if s + 1 < n_groups:
                    load_x(s + 1)
                if s < n_groups:
                    phase_a_mm1(s)
                if 2 <= s:
                    phase_c2a1(s - 2)
                if s < n_groups:
                    phase_a_mm2(s)
                if 2 <= s:
                    phase_c2a2(s - 2)
                # attention for group s-1, interleaved with the previous
                # group's MLP2 big matmuls to keep the PE array HAM-warm
                if 1 <= s <= n_groups:
                    for p in range(4):
                        b_pair(s - 1, p)
                        if 2 <= s:
                            mlp2_chunk(s - 2, p)
                    b_norm(s - 1)
                elif s == n_groups + 1:
                    for p in range(4):
                        mlp2_chunk(s - 2, p)

    nc.compile()
    return nc


_cache = {}


def _get_nc(n_tok):
    if n_tok not in _cache:
        _cache[n_tok] = build(n_tok)
    return _cache[n_tok]


def _prep_in_maps(inputs):
    return _prep(**inputs)


def _prep(x, t_emb, ln1_g, ln1_b, qkv_w, qkv_b, proj_w, proj_b,
          ln2_g, ln2_b, mlp_w1, mlp_b1, mlp_w2, mlp_b2, time_w, time_b):
    x = np.asarray(x, dtype=np.float32)
    t_emb = np.asarray(t_emb, np.float32)
    # host: modulation rows (tiny); fold ln1 gamma/beta + modulation into qkv W/b
    s = t_emb / (1.0 + np.exp(-t_emb))           # silu
    ss = s @ np.asarray(time_w, np.float32) + np.asarray(time_b, np.float32)
    scale, shift = ss[:, :DIM], ss[:, DIM:]
    g1 = np.asarray(ln1_g, np.float32)
    be1 = np.asarray(ln1_b, np.float32)
    arow = g1[None, :] * (1.0 + scale)                      # [B, 512]
    crow = be1[None, :] * (1.0 + scale) + shift             # [B, 512]
    qkv_w = np.asarray(qkv_w, np.float32)
    qkv_b = np.asarray(qkv_b, np.float32)
    # y = xh*arow + crow ; qkv = y@W + b = xh@(diag(arow)W) + (crow@W + b)
    qkvw_eff = arow[:, :, None] * qkv_w[None, :, :]         # [B, 512, 1536]
    qkvb_eff = crow @ qkv_w + qkv_b[None, :]                # [B, 1536]
    # fold ln2 gamma/beta into mlp_w1/b1
    g2 = np.asarray(ln2_g, np.float32)
    be2 = np.asarray(ln2_b, np.float32)
    w1f = np.asarray(mlp_w1, np.float32) * g2[:, None]
    b1f = be2 @ np.asarray(mlp_w1, np.float32) + np.asarray(mlp_b1, np.float32)

    projw16 = np.asarray(proj_w, np.float32).astype(np.float16)
    w116 = w1f.astype(np.float16)
    w216 = np.asarray(mlp_w2, np.float32).astype(np.float16)
    projb = np.asarray(proj_b, np.float32)
    b2 = np.asarray(mlp_b2, np.float32)

    in_maps = []
    nb = x.shape[0]
    for b in range(nb):
        in_maps.append({
            "x": np.ascontiguousarray(x[b]),
            "qkvw": np.ascontiguousarray(qkvw_eff[b].astype(np.float16)),
            "qkvb": np.ascontiguousarray(qkvb_eff[b]),
            "projw": projw16, "projb": projb,
            "w1": w116, "b1": b1f, "w2": w216, "b2": b2,
        })
    return in_maps


def kernel(**inputs):
    in_maps = _prep_in_maps(inputs)
    n_tok = in_maps[0]["x"].shape[0]
    nc = _get_nc(n_tok)
    nb = len(in_maps)
    res = bass_utils.run_bass_kernel_spmd(nc, in_maps, core_ids=list(range(nb)))
    out = np.stack([res.results[b]["out"] for b in range(nb)], axis=0)
    return out


# revision 52
# speedup vs baseline: 1.2151x; 1.1582x over previous
"""Trainium2 Bass kernel for a local-attention transformer block.

Computes, per batch element (one NeuronCore each, 8 cores):
  ss = silu(t_emb) @ time_w + time_b ;  scale, shift = split(ss)
  y  = LN(x) * g1*(1+scale) + (b1*(1+scale)+shift)   (folded host-side into qkv W/b)
  q,k,v = y @ qkv_w + qkv_b  (heads=8, d=64)
  attn: each 128-token window attends to [prev|cur|next] windows
  x1 = x + attn @ proj_w + proj_b
  out = x1 + gelu(LN2(x1) @ w1 + b1') @ w2 + b2       (ln2 g/b host-folded into w1/b1)

Key layout/engine choices (v2):
  - 3-phase software pipeline per 512-token group g: A(g)=LN1+QKV,
    B(g-1)=attention, C(g-2)=proj+MLP; keeps the PE continuously fed.
  - Transposes (y, h2 -> feature-major) via PE matmul against an fp16
    identity (4 per PSUM bank, single evac copy), not DMA xbar.
  - LN rstd = exp(-0.5*ln(var+eps)) so ACT alternates between only the
    natural_log_exp and gelu table sets (2 loads/step instead of ~7).
  - Softmax denominators via ones-column folded into v (row 64 of the AV
    PSUM); normalization broadcast across partitions via a K=2 mask
    matmul instead of DRAM round trips.
  - All matmuls fp16 inputs / fp32 PSUM accumulation.
"""

import numpy as np
import ml_dtypes
from contextlib import ExitStack

import concourse.bass as bass
import concourse.tile as tile
from concourse import bacc, mybir, masks
from concourse import bass_utils

F32 = mybir.dt.float32
F16 = mybir.dt.float16
F8 = mybir.dt.float8e4
DR = mybir.MatmulPerfMode.DoubleRow
AF = mybir.ActivationFunctionType
AL = mybir.AluOpType

DIM = 512
HEADS = 8
HD = 64
FF = 2048
WIN = 128
B = 8
NTOK = 8192
EPS = 1e-5
GRP = 512  # tokens per group (4 windows)


def _bcast_row(dram_ap, offset, n):
    """AP reading dram vector [n] broadcast across 128 partitions."""
    return bass.AP(tensor=dram_ap.tensor, offset=offset, ap=[[0, 128], [1, n]])


def _col_view(dram_ap, offset, ncol):
    """AP reading dram vector [128*ncol] as [128, ncol] feature-major columns."""
    return bass.AP(tensor=dram_ap.tensor, offset=offset, ap=[[1, 128], [128, ncol]])


def build(n_tok=NTOK):
    n_groups = n_tok // GRP
    n_blocks = n_tok // WIN
    nc = bacc.Bacc("TRN2", target_bir_lowering=False, debug=False)

    x_d = nc.dram_tensor("x", [n_tok, DIM], F32, kind="ExternalInput")
    qkvw_d = nc.dram_tensor("qkvw", [DIM, 3 * DIM], F16, kind="ExternalInput")
    qkvb_d = nc.dram_tensor("qkvb", [3 * DIM], F32, kind="ExternalInput")
    projw_d = nc.dram_tensor("projw", [DIM, DIM], F16, kind="ExternalInput")
    projb_d = nc.dram_tensor("projb", [DIM], F32, kind="ExternalInput")
    # w1/w2 pre-paired for DoubleRow: [pair][128, 2*cols] fp8 (plane-major)
    w1_d = nc.dram_tensor("w1", [2, 128, 2 * FF], F8, kind="ExternalInput")
    b1_d = nc.dram_tensor("b1", [FF], F32, kind="ExternalInput")
    w2_d = nc.dram_tensor("w2", [8, 128, 2 * DIM], F8, kind="ExternalInput")
    b2_d = nc.dram_tensor("b2", [DIM], F32, kind="ExternalInput")
    out_d = nc.dram_tensor("out", [n_tok, DIM], F32, kind="ExternalOutput")

    with tile.TileContext(nc) as tc:
        with ExitStack() as ctx:
            consts = ctx.enter_context(tc.tile_pool(name="consts", bufs=1))
            xp = ctx.enter_context(tc.tile_pool(name="xp", bufs=3))
            tp = ctx.enter_context(tc.tile_pool(name="tp", bufs=2))
            yp = ctx.enter_context(tc.tile_pool(name="yp", bufs=1))
            ytp = ctx.enter_context(tc.tile_pool(name="ytp", bufs=1))
            qp = ctx.enter_context(tc.tile_pool(name="qp", bufs=2))
            kp = ctx.enter_context(tc.tile_pool(name="kp", bufs=3))
            vp = ctx.enter_context(tc.tile_pool(name="vp", bufs=3))
            ep = ctx.enter_context(tc.tile_pool(name="ep", bufs=4))
            ap_ = ctx.enter_context(tc.tile_pool(name="ap", bufs=2))
            rp = ctx.enter_context(tc.tile_pool(name="rp", bufs=1))
            x1p = ctx.enter_context(tc.tile_pool(name="x1p", bufs=1))
            h2p = ctx.enter_context(tc.tile_pool(name="h2p", bufs=1))
            h2tp = ctx.enter_context(tc.tile_pool(name="h2tp", bufs=1))
            gp = ctx.enter_context(tc.tile_pool(name="gp", bufs=1))
            op = ctx.enter_context(tc.tile_pool(name="op", bufs=1))
            ps_g = ctx.enter_context(tc.tile_pool(name="ps_g", bufs=2, space="PSUM"))
            ps_tp = ctx.enter_context(tc.tile_pool(name="ps_tp", bufs=1, space="PSUM"))
            ps_s = ctx.enter_context(tc.tile_pool(name="ps_s", bufs=3, space="PSUM"))
            ps_a = ctx.enter_context(tc.tile_pool(name="ps_a", bufs=2, space="PSUM"))

            # ---- x prefetch (group 0 first, ahead of the weight DMAs) ----
            xtiles = {}

            def load_x(g):
                ts_ = []
                for t in range(4):
                    xt = xp.tile([128, DIM], F32, name=f"x_{g}_{t}", tag=f"x{t}")
                    nc.sync.dma_start(xt[:],
                                      x_d[(g * 4 + t) * 128:(g * 4 + t + 1) * 128, :])
                    ts_.append(xt)
                xtiles[g] = ts_

            load_x(0)

            # ---- constants ----
            qkvw_sb = []
            for c in range(4):
                t = consts.tile([128, 3 * DIM], F16, name=f"qkvw{c}", tag=f"qkvw{c}")
                nc.sync.dma_start(t[:], qkvw_d[c * 128:(c + 1) * 128, :])
                qkvw_sb.append(t)
            projw_sb = []
            for c in range(4):
                t = consts.tile([128, DIM], F16, name=f"projw{c}", tag=f"projw{c}")
                nc.sync.dma_start(t[:], projw_d[c * 128:(c + 1) * 128, :])
                projw_sb.append(t)
            w1_sb = []
            for cp in range(2):
                t = consts.tile([128, 2, FF], F8, name=f"w1_{cp}", tag=f"w1_{cp}")
                nc.sync.dma_start(t[:].rearrange("p a b -> p (a b)"), w1_d[cp, :, :])
                w1_sb.append(t)
            w2_sb = []
            for fp in range(8):
                t = consts.tile([128, 2, DIM], F8, name=f"w2_{fp}", tag=f"w2_{fp}")
                nc.sync.dma_start(t[:].rearrange("p a b -> p (a b)"), w2_d[fp, :, :])
                w2_sb.append(t)

            projb_bc = consts.tile([128, DIM], F32, name="projb_bc")
            nc.sync.dma_start(projb_bc[:], _bcast_row(projb_d.ap(), 0, DIM))
            b2_bc = consts.tile([128, DIM], F32, name="b2_bc")
            nc.sync.dma_start(b2_bc[:], _bcast_row(b2_d.ap(), 0, DIM))
            vb_bc = consts.tile([128, DIM], F32, name="vb_bc")
            nc.sync.dma_start(vb_bc[:], _bcast_row(qkvb_d.ap(), 2 * DIM, DIM))
            qkb_sb = consts.tile([128, 8], F32, name="qkb_sb")
            nc.sync.dma_start(qkb_sb[:], _col_view(qkvb_d.ap(), 0, 8))
            b1_sb = consts.tile([128, 16], F32, name="b1_sb")
            nc.sync.dma_start(b1_sb[:], _col_view(b1_d.ap(), 0, 16))
            eps_t = consts.tile([128, 1], F32, name="eps_t")
            nc.vector.memset(eps_t[:], EPS)
            ident = consts.tile([128, 128], F16, name="ident")
            masks.make_identity(nc, ident[:])
            # half-masks: [1,128] fp32 selecting partitions 0-63 / 64-127
            mask_lo = consts.tile([1, 128], F32, name="mask_lo")
            nc.vector.memset(mask_lo[:], 0.0)
            nc.vector.memset(mask_lo[0:1, 0:64], 1.0)
            mask_hi = consts.tile([1, 128], F32, name="mask_hi")
            nc.vector.memset(mask_hi[:], 0.0)
            nc.vector.memset(mask_hi[0:1, 64:128], 1.0)

            stages = {}

            def _rstd4(mvs, tagpfx):
                """rstd[128,4] = 1/sqrt(var+eps) for 4 tiles, DVE-only.

                u = fast_reciprocal(v+eps); r0 = (1+u)/2; then 3 Newton
                iterations r <- r*(1.5 - 0.5*v*r^2). No ACT tables involved.
                """
                v4 = tp.tile([128, 4], F32, name=f"v4_{tagpfx}", tag="v4")
                for t in range(4):
                    nc.vector.tensor_scalar(v4[:, t:t + 1], mvs[t][:, 1:2],
                                            EPS, None, op0=AL.add)
                u = tp.tile([128, 4], F32, name=f"u_{tagpfx}", tag="u4")
                nc.vector.reciprocal_approx_fast(u[:], v4[:])
                r = tp.tile([128, 4], F32, name=f"r_{tagpfx}", tag="r4")
                nc.vector.tensor_scalar(r[:], u[:], 0.5, 0.5,
                                        op0=AL.mult, op1=AL.add)
                t1 = tp.tile([128, 4], F32, name=f"t1_{tagpfx}", tag="t14")
                t2 = tp.tile([128, 4], F32, name=f"t2_{tagpfx}", tag="t24")
                for _ in range(3):
                    nc.vector.tensor_tensor(t1[:], r[:], r[:], op=AL.mult)
                    nc.vector.tensor_tensor(t2[:], t1[:], v4[:], op=AL.mult)
                    nc.vector.tensor_scalar(t1[:], t2[:], -0.5, 1.5,
                                            op0=AL.mult, op1=AL.add)
                    nc.vector.tensor_tensor(r[:], r[:], t1[:], op=AL.mult)
                return r

            def _transpose4(src_tiles, col, dst, psname):
                """dst[128, 512] f16 = concat_t transpose(src_tiles[t][:, col*128:]).

                4 PE identity-matmul transposes into one PSUM bank, one evac.
                """
                P = ps_tp.tile([128, 4, 128], F32, name=psname, tag="tp")
                for t in range(4):
                    nc.tensor.matmul(P[:, t, :],
                                     src_tiles[t][:, col * 128:(col + 1) * 128],
                                     ident[:], start=True, stop=True)
                nc.vector.tensor_copy(dst[:].rearrange("p (a b) -> p a b", a=4), P[:])

            def phase_a_ln(g):
                st = {}
                x_t = xtiles.pop(g)
                y_t, mv_t = [], []
                for t in range(4):
                    stats = tp.tile([128, 6], F32, name=f"st_{g}_{t}", tag=f"st{t}")
                    nc.vector.bn_stats(stats[:], x_t[t][:])
                    mv = tp.tile([128, 2], F32, name=f"mv_{g}_{t}", tag=f"mv{t}")
                    nc.vector.bn_aggr(mv[:], stats[:])
                    mv_t.append(mv)
                rs4 = _rstd4(mv_t, f"a{g}")
                for t in range(4):
                    yt_ = yp.tile([128, DIM], F16, name=f"y_{g}_{t}", tag=f"y{t}")
                    nc.vector.tensor_scalar(yt_[:], x_t[t][:], mv_t[t][:, 0:1],
                                            rs4[:, t:t + 1],
                                            op0=AL.subtract, op1=AL.mult)
                    y_t.append(yt_)
                    # x is only needed for the proj residual; pre-add projb
                    # here, off phase C's critical path.
                    nc.vector.tensor_tensor(x_t[t][:], x_t[t][:], projb_bc[:],
                                            op=AL.add)
                st["x"] = x_t
                st["y"] = y_t
                return st

            def phase_a_mm1(g):
                st = stages[g]
                y_t = st.pop("y")
                yT = []
                for c in range(4):
                    t_ = ytp.tile([128, GRP], F16, name=f"yT_{g}_{c}", tag=f"yT{c}")
                    _transpose4(y_t, c, t_, f"Pt_{g}_{c}")
                    yT.append(t_)
                st["yT"] = yT
                # Q,K feature-major [128 feat, 512 tok], 2 heads per chunk
                q_t, k_t = [], []
                for m in range(8):
                    P = ps_g.tile([128, GRP], F32, name=f"Pqk_{g}_{m}", tag="gemm")
                    for c in range(4):
                        nc.tensor.matmul(P[:], qkvw_sb[c][:, m * 128:(m + 1) * 128],
                                         yT[c][:], start=(c == 0), stop=(c == 3))
                    pool = qp if m < 4 else kp
                    nm = f"q_{g}_{m}" if m < 4 else f"k_{g}_{m-4}"
                    tg = f"q{m}" if m < 4 else f"k{m-4}"
                    sb = pool.tile([128, GRP], F16, name=nm, tag=tg)
                    nc.vector.tensor_scalar(sb[:], P[:], qkb_sb[:, m:m + 1], None,
                                            op0=AL.add)
                    (q_t if m < 4 else k_t).append(sb)
                st["q"], st["k"] = q_t, k_t

            def phase_a_mm2(g):
                st = stages[g]
                yT = st.pop("yT")
                # V token-major with ones column: [128 tok, 8, 65]
                v_t = []
                for t in range(4):
                    P = ps_g.tile([128, DIM], F32, name=f"Pv_{g}_{t}", tag="gemm")
                    for c in range(4):
                        nc.tensor.matmul(P[:], yT[c][:, t * 128:(t + 1) * 128],
                                         qkvw_sb[c][:, 2 * DIM:3 * DIM],
                                         start=(c == 0), stop=(c == 3))
                    vt = vp.tile([128, HEADS, HD + 1], F16, name=f"v_{g}_{t}", tag=f"v{t}")
                    nc.vector.memset(vt[:, :, HD:HD + 1], 1.0)
                    nc.vector.tensor_tensor(
                        vt[:, :, 0:HD],
                        P[:].rearrange("p (h d) -> p h d", h=HEADS),
                        vb_bc[:].rearrange("p (h d) -> p h d", h=HEADS),
                        op=AL.add)
                    v_t.append(vt)
                st["v"] = v_t

            def b_pair(g, c):
                """Attention for heads 2c, 2c+1 — 64-row sim matmuls packed onto
                the two PE array halves via tile_position so they run
                concurrently."""
                cur = stages[g]
                if c == 0:
                    cur["attn"] = [ap_.tile([128, GRP], F16, name=f"attn_{g}_{cc}",
                                            tag=f"attn{cc}") for cc in range(4)]
                    cur["rrec"] = [rp.tile([1, 2, GRP], F32, name=f"rr_{g}_{cc}",
                                           tag=f"rr{cc}") for cc in range(4)]
                    cur["srow"] = [rp.tile([1, 2, GRP], F32, name=f"sr_{g}_{cc}",
                                           tag=f"sr{cc}") for cc in range(4)]
                attn_sb, srow = cur["attn"], cur["srow"]
                P_av = [ps_a.tile([65, GRP], F32, name=f"Pav_{g}_{c}_{hh}", tag="av")
                        for hh in range(2)]
                for wi in range(4):
                    w = 4 * g + wi
                    js = [j for j in (w - 1, w, w + 1) if 0 <= j < n_blocks]
                    P_sim = [ps_s.tile([128, 3, 128], F32, name=f"Ps_{g}_{c}_{wi}_{hh}",
                                       tag="sim") for hh in range(2)]
                    for j in js:
                        d = j - w + 1
                        gj, s = divmod(j, 4)
                        st = stages[gj]
                        for hh, half in ((0, 0), (1, 64)):
                            nc.tensor.matmul(
                                P_sim[hh][:, d, :],
                                st["k"][c][half:half + 64, s * 128:(s + 1) * 128],
                                cur["q"][c][half:half + 64, wi * 128:(wi + 1) * 128],
                                start=True, stop=True, tile_position=(half, 0))
                    dlo, dhi = js[0] - w + 1, js[-1] - w + 1
                    E = [ep.tile([128, 3, 128], F16, name=f"E_{g}_{c}_{wi}_{hh}",
                                 tag="E") for hh in range(2)]
                    for hh in range(2):
                        nc.scalar.activation(E[hh][:, dlo:dhi + 1, :],
                                             P_sim[hh][:, dlo:dhi + 1, :],
                                             AF.Exp, scale=float(HD) ** -0.5)
                    for ji, j in enumerate(js):
                        d = j - w + 1
                        gj, s = divmod(j, 4)
                        st = stages[gj]
                        for hh in range(2):
                            nc.tensor.matmul(
                                P_av[hh][:, wi * 128:(wi + 1) * 128],
                                st["v"][s][:, 2 * c + hh, :],
                                E[hh][:, d, :],
                                start=(ji == 0), stop=(ji == len(js) - 1))
                # unnormalized out rows + stage the sums rows for reciprocal
                for hh, half in ((0, 0), (1, 64)):
                    nc.scalar.activation(attn_sb[c][half:half + 64, :],
                                         P_av[hh][0:64, :], AF.Copy)
                    nc.scalar.activation(srow[c][:, hh, :], P_av[hh][64:65, :],
                                         AF.Copy)

            def b_norm(g):
                cur = stages[g]
                attn_sb, rrec, srow = cur["attn"], cur.pop("rrec"), cur.pop("srow")
                # partition-broadcast of 1/sum via K=1 mask matmuls, then normalize
                for c in range(4):
                    nc.vector.reciprocal_approx_fast(rrec[c][:], srow[c][:])
                    P_rbc = ps_g.tile([128, GRP], F32, name=f"Prb_{g}_{c}", tag="gemm")
                    nc.tensor.matmul(P_rbc[:], mask_lo[:], rrec[c][:, 0, :],
                                     start=True, stop=False)
                    nc.tensor.matmul(P_rbc[:], mask_hi[:], rrec[c][:, 1, :],
                                     start=False, stop=True)
                    nc.vector.tensor_tensor(attn_sb[c][:], attn_sb[c][:], P_rbc[:],
                                            op=AL.mult)

            def phase_c1(g):
                cur = stages[g]
                attn_sb = cur["attn"]
                # proj + residual -> x1 (token-major; projb pre-added into x)
                x1_t, mv2_t = [], []
                for t in range(4):
                    P = ps_g.tile([128, DIM], F32, name=f"Ppr_{g}_{t}", tag="gemm")
                    for c in range(4):
                        nc.tensor.matmul(P[:], attn_sb[c][:, t * 128:(t + 1) * 128],
                                         projw_sb[c][:], start=(c == 0), stop=(c == 3))
                    x1 = x1p.tile([128, DIM], F32, name=f"x1_{g}_{t}", tag=f"x1{t}")
                    nc.vector.tensor_tensor(x1[:], P[:], cur["x"][t][:], op=AL.add)
                    x1_t.append(x1)
                    stats = tp.tile([128, 6], F32, name=f"st2_{g}_{t}", tag=f"st2{t}")
                    nc.vector.bn_stats(stats[:], x1[:])
                    mv = tp.tile([128, 2], F32, name=f"mv2_{g}_{t}", tag=f"mv2{t}")
                    nc.vector.bn_aggr(mv[:], stats[:])
                    mv2_t.append(mv)
                rs4 = _rstd4(mv2_t, f"c{g}")
                # LN2 -> h2 fp16 token-major
                h2_t = []
                for t in range(4):
                    h2 = h2p.tile([128, DIM], F16, name=f"h2_{g}_{t}", tag=f"h2{t}")
                    nc.vector.tensor_scalar(h2[:], x1_t[t][:], mv2_t[t][:, 0:1],
                                            rs4[:, t:t + 1],
                                            op0=AL.subtract, op1=AL.mult)
                    h2_t.append(h2)
                    # x1 is only needed for the mlp residual now; pre-add b2
                    # off the mlp2 critical path.
                    nc.vector.tensor_tensor(x1_t[t][:], x1_t[t][:], b2_bc[:],
                                            op=AL.add)
                cur["x1"], cur["h2"] = x1_t, h2_t
                del cur["x"], cur["attn"]

            def phase_c2a1(g):
                cur = stages[g]
                h2_t = cur.pop("h2")
                # fp8 pair tiles [128, 2, GRP] for DoubleRow mlp1
                h2T = [h2tp.tile([128, 2, GRP], F8, name=f"h2T_{g}_{cp}",
                                 tag=f"h2T{cp}") for cp in range(2)]
                for c in range(4):
                    _transpose4(h2_t, c, h2T[c // 2][:, c % 2, :], f"Pt2_{g}_{c}")
                cur["h2T"] = h2T

            def phase_c2a2(g):
                cur = stages[g]
                h2T = cur.pop("h2T")
                # MLP1 (fp8 DoubleRow, K=256 per matmul) + gelu -> fp8 pairs
                gel = [gp.tile([128, 2, GRP], F8, name=f"gel_{g}_{fp}",
                               tag=f"gel{fp}") for fp in range(8)]
                for f in range(16):
                    P = ps_g.tile([128, GRP], F32, name=f"Pm1_{g}_{f}", tag="gemm")
                    for cp in range(2):
                        nc.tensor.matmul(
                            P[:], w1_sb[cp][:, :, f * 128:(f + 1) * 128],
                            h2T[cp][:], start=(cp == 0), stop=(cp == 1),
                            perf_mode=DR)
                    nc.scalar.activation(gel[f // 2][:, f % 2, :], P[:], AF.Gelu,
                                         bias=b1_sb[:, f:f + 1])
                cur["gel"] = gel

            def mlp2_chunk(g, t):
                # MLP2 (fp8 DoubleRow) + residual -> out (b2 pre-added into x1)
                cur = stages[g]
                gel, x1_t = cur["gel"], cur["x1"]
                P = ps_g.tile([128, DIM], F32, name=f"Pm2_{g}_{t}", tag="gemm")
                for fp in range(8):
                    nc.tensor.matmul(P[:], gel[fp][:, :, t * 128:(t + 1) * 128],
                                     w2_sb[fp][:], start=(fp == 0), stop=(fp == 7),
                                     perf_mode=DR)
                ot = op.tile([128, DIM], F32, name=f"o_{g}_{t}", tag=f"o{t}")
                nc.vector.tensor_tensor(ot[:], P[:], x1_t[t][:], op=AL.add)
                nc.sync.dma_start(out_d[(g * 4 + t) * 128:(g * 4 + t + 1) * 128, :],
                                  ot[:])
                if t == 3:
                    del cur["gel"], cur["x1"]

            for s in range(n_groups + 2):
                if s < n_groups:
                    stages[s] = phase_a_ln(s)
                if 2 <= s:
                    phase_c1(s - 2)
                if s + 1 < n_groups:
                    load_x(s + 1)
                if s < n_groups:
                    phase_a_mm1(s)
                if 2 <= s:
                    phase_c2a1(s - 2)
                if s < n_groups:
                    phase_a_mm2(s)
                if 2 <= s:
                    phase_c2a2(s - 2)
                # attention for group s-1, interleaved with the previous
                # group's MLP2 big matmuls to keep the PE array HAM-warm
                if 1 <= s <= n_groups:
                    for p in range(4):
                        b_pair(s - 1, p)
                        if 2 <= s:
                            mlp2_chunk(s - 2, p)
                    b_norm(s - 1)
                elif s == n_groups + 1:
                    for p in range(4):
                        mlp2_chunk(s - 2, p)

    nc.compile()
    return nc


_cache = {}


def _get_nc(n_tok):
    if n_tok not in _cache:
        _cache[n_tok] = build(n_tok)
    return _cache[n_tok]


def _prep_in_maps(inputs):
    return _prep(**inputs)


def _prep(x, t_emb, ln1_g, ln1_b, qkv_w, qkv_b, proj_w, proj_b,
          ln2_g, ln2_b, mlp_w1, mlp_b1, mlp_w2, mlp_b2, time_w, time_b):
    x = np.asarray(x, dtype=np.float32)
    t_emb = np.asarray(t_emb, np.float32)
    # host: modulation rows (tiny); fold ln1 gamma/beta + modulation into qkv W/b
    s = t_emb / (1.0 + np.exp(-t_emb))           # silu
    ss = s @ np.asarray(time_w, np.float32) + np.asarray(time_b, np.float32)
    scale, shift = ss[:, :DIM], ss[:, DIM:]
    g1 = np.asarray(ln1_g, np.float32)
    be1 = np.asarray(ln1_b, np.float32)
    arow = g1[None, :] * (1.0 + scale)                      # [B, 512]
    crow = be1[None, :] * (1.0 + scale) + shift             # [B, 512]
    qkv_w = np.asarray(qkv_w, np.float32)
    qkv_b = np.asarray(qkv_b, np.float32)
    # y = xh*arow + crow ; qkv = y@W + b = xh@(diag(arow)W) + (crow@W + b)
    qkvw_eff = arow[:, :, None] * qkv_w[None, :, :]         # [B, 512, 1536]
    qkvb_eff = crow @ qkv_w + qkv_b[None, :]                # [B, 1536]
    # fold ln2 gamma/beta into mlp_w1/b1
    g2 = np.asarray(ln2_g, np.float32)
    be2 = np.asarray(ln2_b, np.float32)
    w1f = np.asarray(mlp_w1, np.float32) * g2[:, None]
    b1f = be2 @ np.asarray(mlp_w1, np.float32) + np.asarray(mlp_b1, np.float32)

    projw16 = np.asarray(proj_w, np.float32).astype(np.float16)
    # w1/w2 pre-paired fp8 for DoubleRow: [pair][128, 2*cols], plane-major
    FP8 = ml_dtypes.float8_e4m3fn
    w18 = np.ascontiguousarray(
        w1f.reshape(2, 2, 128, FF).transpose(0, 2, 1, 3).reshape(2, 128, 2 * FF)
    ).astype(FP8)
    w28 = np.ascontiguousarray(
        np.asarray(mlp_w2, np.float32)
        .reshape(8, 2, 128, DIM).transpose(0, 2, 1, 3).reshape(8, 128, 2 * DIM)
    ).astype(FP8)
    projb = np.asarray(proj_b, np.float32)
    b2 = np.asarray(mlp_b2, np.float32)

    in_maps = []
    nb = x.shape[0]
    for b in range(nb):
        in_maps.append({
            "x": np.ascontiguousarray(x[b]),
            "qkvw": np.ascontiguousarray(qkvw_eff[b].astype(np.float16)),
            "qkvb": np.ascontiguousarray(qkvb_eff[b]),
            "projw": projw16, "projb": projb,
            "w1": w18, "b1": b1f, "w2": w28, "b2": b2,
        })
    return in_maps


def kernel(**inputs):
    in_maps = _prep_in_maps(inputs)
    n_tok = in_maps[0]["x"].shape[0]
    nc = _get_nc(n_tok)
    nb = len(in_maps)
    res = bass_utils.run_bass_kernel_spmd(nc, in_maps, core_ids=list(range(nb)))
    out = np.stack([res.results[b]["out"] for b in range(nb)], axis=0)
    return out
